# revision 1
# baseline (speedup 1.0000x reference)
"""Trainium2 Bass kernel for nn_APG_MLP_Layer (3-layer APG hyper-network MLP).

Reference computation per layer (B=8192, din=dout=1024, RANK=64):
    w = (x @ Wm.T + bm).reshape(B, 64, 64)   # per-sample generated weights
    u = x @ U.T                              # [B, 64]
    h = einsum('br,brs->bs', u, w)           # per-sample vec-mat product
    out = relu?(h @ V.T + b)

Sharding: data-parallel over batch across 8 NeuronCores (1024 rows/core);
static params replicated.

Device mapping (per core, per 128-row batch tile):
  - Wm GEMM dominates (8192x1024x4096 per layer). Host pre-transposes all
    static operands and reorders Wm rows to j' = s*64 + r so that each PSUM
    chunk [128b, 512] holds w'[b, s_block(8), r(64)] with r contiguous.
  - The einsum contraction is then one DVE tensor_tensor multiply with u
    broadcast over s (step-0 AP) + one inner-axis tensor_reduce -> h[b, s].
  - h is PE-transposed; layers 0/1 compute the V GEMM in transposed form
    outT[o, b] = V.T^T(slice) @ hT so the ReLU'd output is directly the
    next layer's lhsT (k on partitions). Layer 2 computes out[b, o].
  - All matmuls run in bf16 (fp32 accumulate in PSUM).

The kernel has a runtime `reps` loop (register-bound For_i) so the same NEFF
serves correctness (reps=1) and steady-state timing (reps=R, slope method).
"""

import numpy as np
import ml_dtypes

import concourse.bass as bass
import concourse.mybir as mybir
from concourse import bacc
from concourse.tile import TileContext
from concourse.masks import make_identity

BF16 = ml_dtypes.bfloat16
FP32 = mybir.dt.float32
BF = mybir.dt.bfloat16

B = 8192
NCORES = 8
BL = B // NCORES          # 1024 rows per core
D = 1024                  # all layer dims
R = 64                    # rank
NBT = BL // 128           # batch tiles per core (8)
NK = D // 128             # k chunks (8)
NJ = (R * R) // 512       # j chunks of 512 (8)
NOC = D // 128            # output chunks (8)


def build_apg(include_bm=False, include_b01=False, include_b2=False,
              reps_loop=True, pipeline=False, u_fold=True, v_dma_t=False,
              h_dma_t=False, wm_gp=False, v_batch=False, ablate=None,
              loop_kwargs=None, tmp_bufs=3, wm_bufs=16, act_bufs=2,
              wm_n=512, pw_bufs=None, persist=(0, 0, 0), persist_xt=False,
              osb_bufs=2, v_pack=False, po_bufs=None):
    """Build + compile the Bass module. Returns (nc, names) where names lists
    the DRAM input tensor names in declaration order."""
    import contextlib
    wm_nodma = ablate == "pe_wm_nodma"
    if wm_nodma:
        ablate = "pe_wm"
    if v_pack:
        v_batch = True
    nc = bacc.Bacc("TRN2", target_bir_lowering=False, debug=False,
                   num_devices=NCORES)

    xt = nc.dram_tensor("xt", [D, BL], BF, kind="ExternalInput")
    wmt = [nc.dram_tensor(f"wmt{l}", [D, R * R], BF, kind="ExternalInput")
           for l in range(3)]
    ut = [nc.dram_tensor(f"ut{l}", [D, R], BF, kind="ExternalInput")
          for l in range(3)]
    vt = [nc.dram_tensor(f"vt{l}", [R, D], BF, kind="ExternalInput")
          for l in range(3)]
    bm_row = b01_col = b2_row = None
    if include_bm:
        bm_row = [nc.dram_tensor(f"bmr{l}", [1, R * R], BF, kind="ExternalInput")
                  for l in range(3)]
    if include_b01:
        b01_col = [nc.dram_tensor(f"b{l}c", [128, NOC], FP32, kind="ExternalInput")
                   for l in range(2)]
    if include_b2:
        b2_row = nc.dram_tensor("b2r", [1, D], BF, kind="ExternalInput")
    reps_t = None
    if reps_loop:
        reps_t = nc.dram_tensor("reps", [1, 1], mybir.dt.uint32,
                                kind="ExternalInput")
    out_d = nc.dram_tensor("out", [BL, D], FP32, kind="ExternalOutput")

    with TileContext(nc) as tc:
        with (
            tc.tile_pool(name="const", bufs=1) as constp,
            tc.tile_pool(name="xt", bufs=2) as xtp,
            tc.tile_pool(name="wm", bufs=wm_bufs) as wmp,
            tc.tile_pool(name="usb", bufs=act_bufs) as usbp,
            tc.tile_pool(name="h", bufs=act_bufs) as hp,
            tc.tile_pool(name="ht", bufs=act_bufs) as htp,
            tc.tile_pool(name="tmp", bufs=tmp_bufs) as tmpp,
            tc.tile_pool(name="osb", bufs=osb_bufs) as osbp,
            tc.tile_pool(name="pw", bufs=(pw_bufs if pw_bufs is not None
                                          else (3 if v_pack else 4)
                                          if wm_n == 512 else 2),
                         space="PSUM") as pwp,
            tc.tile_pool(name="pu", bufs=1, space="PSUM") as pup,
            tc.tile_pool(name="pt", bufs=1, space="PSUM") as ptp,
            tc.tile_pool(name="po", bufs=(po_bufs if po_bufs is not None
                                          else 3 if v_pack
                                          else 2 if v_batch else 1),
                         space="PSUM") as pop,
            tc.tile_pool(name="ht4", bufs=2) as ht4p,
        ):
            # ---- constants (loaded once, outside the reps loop) ----
            ident = constp.tile([128, 128], FP32, tag="ident")
            make_identity(nc, ident[:, :])
            vt_sb = []
            vt2_sb = []
            for l in range(3):
                if v_pack:
                    # V.T duplicated on both partition halves so K=64 V-GEMMs
                    # can run pairwise in disjoint PE row groups
                    t2 = constp.tile([128, D], BF, tag=f"vt2_{l}")
                    nc.sync.dma_start(out=t2[0:R, :], in_=vt[l][:, :])
                    nc.sync.dma_start(out=t2[R:128, :], in_=vt[l][:, :])
                    vt2_sb.append(t2)
                    vt_sb.append(t2)
                else:
                    t = constp.tile([R, D], BF, tag=f"vt{l}")
                    nc.sync.dma_start(out=t[:, :], in_=vt[l][:, :])
                    vt_sb.append(t)
                    vt2_sb.append(None)
            ut_sb = []
            for l in range(3):
                # [128, NK*R]: column block k holds U_l.T rows k*128..k*128+127
                t = constp.tile([128, NK * R], BF, tag=f"ut{l}")
                nc.sync.dma_start(
                    out=t[:, :].rearrange("p (k r) -> p k r", r=R),
                    in_=ut[l][:, :].rearrange("(k p) r -> p k r", p=128))
                ut_sb.append(t)
            ones_bf = None
            if include_bm or include_b2:
                ones_bf = constp.tile([1, 128], BF, tag="ones")
                nc.vector.memset(ones_bf[:, :], 1.0)
            bmr_sb = []
            if include_bm:
                for l in range(3):
                    t = constp.tile([1, R * R], BF, tag=f"bmr{l}")
                    nc.sync.dma_start(out=t[:, :], in_=bm_row[l][:, :])
                    bmr_sb.append(t)
            b01_sb = []
            if include_b01:
                for l in range(2):
                    t = constp.tile([128, NOC], FP32, tag=f"b01_{l}")
                    nc.sync.dma_start(out=t[:, :], in_=b01_col[l][:, :])
                    b01_sb.append(t)
            b2_sb = None
            if include_b2:
                b2_sb = constp.tile([1, D], BF, tag="b2")
                nc.sync.dma_start(out=b2_sb[:, :], in_=b2_row[:, :])

            # runtime rep count on all engines
            if reps_loop:
                regs = nc.alloc_registers("reps_regs", mybir.ALL_ENGINES)
                nc.regs_load(regs, reps_t[0:1, 0:1])
                reps_val = nc.snap(regs, donate=True, min_val=1, max_val=1 << 20)
                loop_cm = tc.For_i(0, reps_val, 1, **(loop_kwargs or {}))
            else:
                loop_cm = contextlib.nullcontext()

            wm_static = None
            if wm_nodma:
                # one wm tile set loaded outside the reps loop, reused for
                # all layers (timing ablation only — results are wrong)
                wm_static = []
                for k in range(NK):
                    t = constp.tile([128, R * R], BF, tag=f"wmstat{k}")
                    nc.sync.dma_start(out=t[:, :],
                                      in_=wmt[0][k * 128:(k + 1) * 128, :])
                    wm_static.append(t)

            # weight-stationary: persist the first persist[l] wm tiles of each
            # layer in SBUF (loaded once, outside the reps loop)
            wm_persist = {}
            for l in range(3):
                for k in range(persist[l]):
                    t = constp.tile([128, R * R], BF, tag=f"wmp{l}_{k}")
                    nc.sync.dma_start(out=t[:, :],
                                      in_=wmt[l][k * 128:(k + 1) * 128, :])
                    wm_persist[(l, k)] = t
            xt_static = None
            if persist_xt:
                xt_static = constp.tile([128, NK * BL], BF, tag="xt0")
                nc.sync.dma_start(
                    out=xt_static[:, :].rearrange("p (k b) -> p k b", b=BL),
                    in_=xt[:, :].rearrange("(k p) b -> p k b", p=128))

            with loop_cm:
                # activations (lhsT layout): [128, NK*BL] bf16; col block k
                # holds x.T rows k*128..k*128+127 (i.e. x cols), b along free.
                if persist_xt:
                    xt_cur = xt_static
                else:
                    xt_cur = xtp.tile([128, NK * BL], BF, tag="act")
                    nc.sync.dma_start(
                        out=xt_cur[:, :].rearrange("p (k b) -> p k b", b=BL),
                        in_=xt[:, :].rearrange("(k p) b -> p k b", p=128))

                # Software pipeline over (layer, batch-tile): each
                # iteration's tail (h transpose + V GEMM + relu/output) is
                # emitted interleaved into the NEXT iteration's wm-GEMM
                # stream so its small LDWEIGHTS-bound matmuls hide behind
                # the 512-column wm matmuls. `pending` holds the tail
                # closures of the previous (l, bt).
                pending = []

                def emit_slot():
                    if pending:
                        pending.pop(0)()

                def make_tail(l, bt, h_sb, xt_next, bsl):
                    vt_t = vt_sb[l]
                    items = []

                    if h_dma_t:
                        # keep the transpose off the PE: cast h to bf16 on
                        # DVE, transpose via the DMA xbar. The xbar wants
                        # 128x128 tiles, so pad: only cols 0:64 of h_pad are
                        # written and only rows 0:64 of ht_pad are read.
                        h_pad = hp.tile([128, 128], BF, tag="h_bf")
                        ht_pad = htp.tile([128, 128], BF, tag="ht_pad")
                        ht_sb = ht_pad[0:R, :]

                        def t_transpose():
                            nc.vector.tensor_copy(out=h_pad[:, 0:R], in_=h_sb[:, :])
                            nc.sync.dma_start_transpose(out=ht_pad[:, :],
                                                        in_=h_pad[:, :])
                    else:
                        pt = ptp.tile([R, 128], FP32)
                        ht_tile = htp.tile([R, 128], BF)
                        ht_sb = ht_tile[:, :]

                        def t_transpose():
                            nc.tensor.transpose(pt[:, :], h_sb[:, :], ident[:, :])
                            nc.scalar.copy(ht_sb, pt[:, :])
                    items.append(t_transpose)

                    if l < 2 and v_dma_t:
                        # V GEMM in [b, o] layout with hT stationary (one
                        # LDWEIGHTS), relu to bf16, then DMA-xbar transpose
                        # each [128,128] block into the next layer's lhsT.
                        po = pop.tile([128, D], FP32)
                        for half in range(2):
                            def t_v2a(half=half):
                                osl = slice(half * 512, (half + 1) * 512)
                                nc.tensor.matmul(
                                    po[:, osl], ht_sb[:, :], vt_t[:, osl],
                                    start=True, stop=True)
                            items.append(t_v2a)

                        def t_relu_t():
                            o_bf = osbp.tile([128, D], BF, tag="obf")
                            nc.scalar.activation(
                                o_bf[:, :], po[:, :],
                                mybir.ActivationFunctionType.Relu)
                            for oc in range(NOC):
                                nc.sync.dma_start_transpose(
                                    out=xt_next[:, oc * BL + bt * 128:
                                                oc * BL + bt * 128 + 128],
                                    in_=o_bf[:, oc * 128:(oc + 1) * 128])
                        items.append(t_relu_t)
                    elif l < 2:
                        po = pop.tile([128, NOC * 128], FP32)
                        for oc in range(NOC):
                            def t_v(oc=oc):
                                nc.tensor.matmul(
                                    po[:, oc * 128:(oc + 1) * 128],
                                    vt_t[:, oc * 128:(oc + 1) * 128],
                                    ht_sb[:, :], start=True, stop=True)
                            items.append(t_v)

                        def t_relu():
                            if include_b01:
                                for oc in range(NOC):
                                    nc.scalar.activation(
                                        xt_next[:, oc * BL + bt * 128:
                                                oc * BL + bt * 128 + 128],
                                        po[:, oc * 128:(oc + 1) * 128],
                                        mybir.ActivationFunctionType.Relu,
                                        bias=b01_sb[l][:, oc:oc + 1], scale=1.0)
                            else:
                                nc.scalar.activation(
                                    xt_next[:, :]
                                    .rearrange("p (k b) -> p k b", b=BL)
                                    [:, :, bt * 128:bt * 128 + 128],
                                    po[:, :].rearrange("p (k c) -> p k c", c=128),
                                    mybir.ActivationFunctionType.Relu)
                        items.append(t_relu)
                    else:
                        po = pop.tile([128, D], FP32)
                        for half in range(2):
                            def t_v2(half=half):
                                osl = slice(half * 512, (half + 1) * 512)
                                nc.tensor.matmul(
                                    po[:, osl], ht_sb[:, :], vt_t[:, osl],
                                    start=True, stop=not include_b2)
                                if include_b2:
                                    nc.tensor.matmul(
                                        po[:, osl], ones_bf[:, :], b2_sb[:, osl],
                                        start=False, stop=True)
                            items.append(t_v2)

                        def t_out():
                            o_sb = osbp.tile([128, D], FP32)
                            nc.scalar.copy(o_sb[:, :], po[:, :])
                            nc.sync.dma_start(out=out_d[bsl, :], in_=o_sb[:, :])
                        items.append(t_out)
                    return items

                wm_tiles = None
                xt_l = xt_cur
                for l in range(3):
                    if wm_nodma:
                        wm_tiles = wm_static
                    else:
                        wm_tiles = []
                        dma_eng = nc.gpsimd if wm_gp else nc.sync
                        for k in range(NK):
                            if (l, k) in wm_persist:
                                wm_tiles.append(wm_persist[(l, k)])
                                continue
                            t = wmp.tile([128, R * R], BF, tag="wm")
                            dma_eng.dma_start(out=t[:, :],
                                              in_=wmt[l][k * 128:(k + 1) * 128, :])
                            wm_tiles.append(t)
                    xt_next = None
                    if l < 2 and ablate != "pe_wm":
                        xt_next = xtp.tile([128, NK * BL], BF, tag="act")

                    for bt in range(NBT):
                        bsl = slice(bt * 128, (bt + 1) * 128)
                        xt_b = xt_l

                        def lhs(k, xt_b=xt_b, bt=bt):
                            return xt_b[:, k * BL + bt * 128:
                                        k * BL + bt * 128 + 128]

                        pu = u_sb = h_sb = None
                        if ablate != "pe_wm":
                            pu = pup.tile([128, R], FP32)
                            u_sb = usbp.tile([128, R], FP32)
                            h_sb = hp.tile([128, R], FP32)
                        if not u_fold:
                            for k in range(NK):
                                nc.tensor.matmul(pu[:, :], lhs(k),
                                                 ut_sb[l][:, k * R:(k + 1) * R],
                                                 start=(k == 0),
                                                 stop=(k == NK - 1))
                            nc.scalar.copy(u_sb[:, :], pu[:, :])
                        nj = (R * R) // wm_n
                        s_per = wm_n // R
                        for j in range(nj):
                            pw = pwp.tile([128, wm_n], FP32)
                            for k in range(NK):
                                wm_mv = wm_tiles[k][:, j * wm_n:(j + 1) * wm_n]
                                pw_out = pw[:, :]
                                if wm_n > 512:
                                    # ISA caps a single AP dim at 512 elements
                                    wm_mv = wm_mv.rearrange(
                                        "p (t n) -> p t n", n=512)
                                    pw_out = pw_out.rearrange(
                                        "p (t n) -> p t n", n=512)
                                nc.tensor.matmul(
                                    pw_out, lhs(k), wm_mv,
                                    start=(k == 0),
                                    stop=(k == NK - 1) and not include_bm)
                                if j == 0 and u_fold and ablate != "pe_wm":
                                    # u GEMM rides the j=0 chunk so its
                                    # LDWEIGHTS hide behind wide matmuls
                                    nc.tensor.matmul(
                                        pu[:, :], lhs(k),
                                        ut_sb[l][:, k * R:(k + 1) * R],
                                        start=(k == 0), stop=(k == NK - 1))
                            if include_bm:
                                nc.tensor.matmul(
                                    pw[:, :], ones_bf[:, :],
                                    bmr_sb[l][:, j * wm_n:(j + 1) * wm_n],
                                    start=False, stop=True)
                            if j == 0 and u_fold and ablate != "pe_wm":
                                nc.scalar.copy(u_sb[:, :], pu[:, :])
                            if ablate == "nodve":
                                nc.vector.tensor_copy(
                                    out=h_sb[:, j * s_per:(j + 1) * s_per],
                                    in_=pw[:, 0:s_per])
                            if ablate != "nodve" and ablate != "pe_wm":
                                # tmp[b, s, r] = w'[b, s, r] * u[b, r]
                                tmp = tmpp.tile([128, wm_n], FP32)
                                nc.vector.tensor_tensor(
                                    out=tmp[:, :].rearrange("p (s r) -> p s r", r=R),
                                    in0=pw[:, :].rearrange("p (s r) -> p s r", r=R),
                                    in1=u_sb[:, :].unsqueeze(1)
                                    .broadcast_to([128, s_per, R]),
                                    op=mybir.AluOpType.mult)
                                # h[b, jc*s_per + s] = sum_r tmp[b, s, r]
                                nc.vector.tensor_reduce(
                                    out=h_sb[:, j * s_per:(j + 1) * s_per],
                                    in_=tmp[:, :].rearrange("p (s r) -> p s r", r=R),
                                    axis=mybir.AxisListType.X,
                                    op=mybir.AluOpType.add)
                            for _ in range(wm_n // 512):
                                emit_slot()
                            if j == 0 or j == nj - 1:
                                emit_slot()
                                emit_slot()
                        assert not pending, "tail did not fit in slot budget"
                        if v_batch and ablate is None:
                            # transpose h into the 4-tile group's shared hT
                            # operand; run the V GEMM once per group at
                            # n=512 so LDWEIGHTS amortizes.
                            if bt % 4 == 0:
                                ht4 = ht4p.tile([128 if v_pack else R, 512],
                                                BF, tag="ht4")
                            pt = ptp.tile([R, 128], FP32)
                            nc.tensor.transpose(pt[:, :], h_sb[:, :], ident[:, :])
                            nc.scalar.copy(
                                ht4[0:R, (bt % 4) * 128:(bt % 4 + 1) * 128],
                                pt[:, :])
                            if v_pack:
                                # second copy on partitions 64..127 feeds the
                                # row-group-packed partner matmul
                                nc.scalar.copy(
                                    ht4[R:128, (bt % 4) * 128:(bt % 4 + 1) * 128],
                                    pt[:, :])
                            if bt % 4 == 3:
                                g0 = (bt - 3) * 128
                                if l < 2 and v_pack:
                                    for ocp in range(NOC // 2):
                                        oc0, oc1 = 2 * ocp, 2 * ocp + 1
                                        po0 = pop.tile([128, 512], FP32, tag="po")
                                        po1 = pop.tile([128, 512], FP32, tag="po")
                                        nc.tensor.matmul(
                                            po0[:, :],
                                            vt2_sb[l][0:R, oc0 * 128:(oc0 + 1) * 128],
                                            ht4[0:R, :], start=True, stop=True)
                                        nc.tensor.matmul(
                                            po1[:, :],
                                            vt2_sb[l][R:128, oc1 * 128:(oc1 + 1) * 128],
                                            ht4[R:128, :], start=True, stop=True)
                                        for oc, po in ((oc0, po0), (oc1, po1)):
                                            dst = xt_next[:, oc * BL + g0:
                                                          oc * BL + g0 + 512]
                                            if include_b01:
                                                nc.scalar.activation(
                                                    dst, po[:, :],
                                                    mybir.ActivationFunctionType.Relu,
                                                    bias=b01_sb[l][:, oc:oc + 1],
                                                    scale=1.0)
                                            elif oc % 2 == 1:
                                                # split relus across ACT and DVE
                                                nc.vector.tensor_scalar(
                                                    out=dst, in0=po[:, :],
                                                    scalar1=0.0,
                                                    op0=mybir.AluOpType.max)
                                            else:
                                                nc.scalar.activation(
                                                    dst, po[:, :],
                                                    mybir.ActivationFunctionType.Relu)
                                elif l < 2:
                                    for oc in range(NOC):
                                        po = pop.tile([128, 512], FP32, tag="po")
                                        nc.tensor.matmul(
                                            po[:, :],
                                            vt_sb[l][:, oc * 128:(oc + 1) * 128],
                                            ht4[:, :], start=True, stop=True)
                                        if include_b01:
                                            nc.scalar.activation(
                                                xt_next[:, oc * BL + g0:
                                                        oc * BL + g0 + 512],
                                                po[:, :],
                                                mybir.ActivationFunctionType.Relu,
                                                bias=b01_sb[l][:, oc:oc + 1],
                                                scale=1.0)
                                        else:
                                            nc.scalar.activation(
                                                xt_next[:, oc * BL + g0:
                                                        oc * BL + g0 + 512],
                                                po[:, :],
                                                mybir.ActivationFunctionType.Relu)
                                elif v_pack:
                                    assert not include_b2
                                    for i4 in range(4):
                                        o_sb = osbp.tile([128, D], FP32)
                                        po0 = pop.tile([128, 512], FP32, tag="po")
                                        po1 = pop.tile([128, 512], FP32, tag="po")
                                        nc.tensor.matmul(
                                            po0[:, :],
                                            ht4[0:R, i4 * 128:(i4 + 1) * 128],
                                            vt2_sb[l][0:R, 0:512],
                                            start=True, stop=True)
                                        nc.tensor.matmul(
                                            po1[:, :],
                                            ht4[R:128, i4 * 128:(i4 + 1) * 128],
                                            vt2_sb[l][R:128, 512:1024],
                                            start=True, stop=True)
                                        nc.scalar.copy(o_sb[:, 0:512], po0[:, :])
                                        nc.vector.tensor_copy(
                                            out=o_sb[:, 512:1024], in_=po1[:, :])
                                        row0 = (bt - 3 + i4) * 128
                                        nc.sync.dma_start(
                                            out=out_d[row0:row0 + 128, :],
                                            in_=o_sb[:, :])
                                else:
                                    for i4 in range(4):
                                        o_sb = osbp.tile([128, D], FP32)
                                        for half in range(2):
                                            osl = slice(half * 512,
                                                        (half + 1) * 512)
                                            po = pop.tile([128, 512], FP32,
                                                          tag="po")
                                            nc.tensor.matmul(
                                                po[:, :],
                                                ht4[:, i4 * 128:(i4 + 1) * 128],
                                                vt_sb[l][:, osl],
                                                start=True, stop=not include_b2)
                                            if include_b2:
                                                nc.tensor.matmul(
                                                    po[:, :], ones_bf[:, :],
                                                    b2_sb[:, osl],
                                                    start=False, stop=True)
                                            nc.scalar.copy(o_sb[:, osl],
                                                           po[:, :])
                                        row0 = (bt - 3 + i4) * 128
                                        nc.sync.dma_start(
                                            out=out_d[row0:row0 + 128, :],
                                            in_=o_sb[:, :])
                            continue
                        if ablate == "pe_wm":
                            if l == 2 and bt == NBT - 1:
                                # keep the output write so the graph has one
                                nc.sync.dma_start(
                                    out=out_d[bsl, :],
                                    in_=xt_b[:, :].bitcast(FP32)[:, 0:D])
                            continue
                        pending = make_tail(l, bt, h_sb, xt_next, bsl)
                        if not pipeline:
                            for fn in pending:
                                fn()
                            pending = []
                    if xt_next is not None:
                        xt_l = xt_next

                for fn in pending:
                    fn()
                pending = []
    nc.compile()
    return nc


# ---------------------------------------------------------------------------
# host side
# ---------------------------------------------------------------------------

def _prep_static(Wm, bm, U, V, b):
    """Host-side layout prep of one layer's replicated params."""
    Wm = np.asarray(Wm, dtype=np.float32)
    # rows j' = s*64 + r  <->  original j = r*64 + s ; then transpose -> [k, j']
    wmt = np.ascontiguousarray(
        Wm.reshape(R, R, D).transpose(1, 0, 2).reshape(R * R, D).T).astype(BF16)
    utm = np.ascontiguousarray(np.asarray(U, dtype=np.float32).T).astype(BF16)
    vtm = np.ascontiguousarray(np.asarray(V, dtype=np.float32).T).astype(BF16)
    return wmt, utm, vtm


_CACHE = {}


def _get_compiled(flags):
    if flags not in _CACHE:
        _CACHE[flags] = build_apg(*flags)
    return _CACHE[flags]


def _make_in_maps(x, layers, flags, reps):
    include_bm, include_b01, include_b2 = flags
    x = np.asarray(x, dtype=np.float32)
    shared = {}
    for l, (Wm, bm, U, V, b) in enumerate(layers):
        wmt, utm, vtm = _prep_static(Wm, bm, U, V, b)
        shared[f"wmt{l}"] = wmt
        shared[f"ut{l}"] = utm
        shared[f"vt{l}"] = vtm
        if include_bm:
            shared[f"bmr{l}"] = np.asarray(bm, np.float32).reshape(R, R).T \
                .reshape(1, R * R).astype(BF16)
        if include_b01 and l < 2:
            shared[f"b{l}c"] = np.ascontiguousarray(
                np.asarray(b, np.float32).reshape(NOC, 128).T)
        if include_b2 and l == 2:
            shared["b2r"] = np.asarray(b, np.float32).reshape(1, D).astype(BF16)
    shared["reps"] = np.array([[reps]], dtype=np.uint32)
    in_maps = []
    for i in range(NCORES):
        m = dict(shared)
        xs = x[i * BL:(i + 1) * BL, :]
        m["xt"] = np.ascontiguousarray(xs.T).astype(BF16)
        in_maps.append(m)
    return in_maps


_RUNNER_CACHE = {}


def _get_runner(flags):
    """Jit-once PJRT runner for the compiled module (same execution path as
    bass_utils.run_bass_kernel_spmd's axon redirect through bass2jax, but
    cached so repeat kernel() calls skip re-trace/re-compile)."""
    if flags in _RUNNER_CACHE:
        return _RUNNER_CACHE[flags]
    import jax
    from jax.sharding import Mesh, PartitionSpec, NamedSharding
    from jax.experimental.shard_map import shard_map
    from concourse import bass2jax

    nc = _get_compiled(flags)
    bass2jax.install_neuronx_cc_hook()
    partition_name = nc.partition_id_tensor.name if nc.partition_id_tensor else None
    in_names, out_names, out_avals, zero_outs = [], [], [], []
    for alloc in nc.m.functions[0].allocations:
        if not isinstance(alloc, mybir.MemoryLocationSet):
            continue
        name = alloc.memorylocations[0].name
        if alloc.kind == "ExternalInput":
            if name != partition_name:
                in_names.append(name)
        elif alloc.kind == "ExternalOutput":
            out_names.append(name)
            shape = tuple(alloc.tensor_shape)
            dtype = mybir.dt.np(alloc.dtype)
            out_avals.append(jax.core.ShapedArray(shape, dtype))
            zero_outs.append(np.zeros(shape, dtype))
    n_params = len(in_names)
    all_in_names = list(in_names) + list(out_names)
    if partition_name is not None:
        all_in_names.append(partition_name)

    def _body(*args):
        operands = list(args)
        if partition_name is not None:
            operands = operands + [bass2jax.partition_id_tensor()]
        outs = bass2jax._bass_exec_p.bind(
            *operands, out_avals=tuple(out_avals), in_names=tuple(all_in_names),
            out_names=tuple(out_names), lowering_input_output_aliases=(),
            sim_require_finite=True, sim_require_nnan=True, nc=nc)
        return tuple(outs)

    devices = jax.devices()[:NCORES]
    mesh = Mesh(np.asarray(devices), ("core",))
    in_specs = (PartitionSpec("core"),) * (n_params + len(out_names))
    out_specs = (PartitionSpec("core"),) * len(out_names)
    fn = jax.jit(shard_map(_body, mesh=mesh, in_specs=in_specs,
                           out_specs=out_specs, check_rep=False))
    sh = NamedSharding(mesh, PartitionSpec("core"))

    # Outputs are constant zero-filled donor buffers — upload once.
    zero_dev = [jax.device_put(np.concatenate([z] * NCORES, axis=0), sh)
                for z in zero_outs]
    dev_cache = {}

    def _put_cached(name, arr):
        import hashlib
        key = (name, arr.shape, arr.dtype.str,
               hashlib.blake2b(np.ascontiguousarray(arr).tobytes(),
                               digest_size=16).hexdigest())
        if key not in dev_cache:
            if len(dev_cache) > 64:
                dev_cache.clear()
            dev_cache[key] = jax.device_put(arr, sh)
        return dev_cache[key]

    def run(in_maps):
        dev = [_put_cached(name,
                           np.concatenate([np.asarray(m[name]) for m in in_maps],
                                          axis=0))
               for name in in_names]
        outs = fn(*(dev + zero_dev))
        jax.block_until_ready(outs)
        return {name: np.asarray(outs[i]) for i, name in enumerate(out_names)}

    _RUNNER_CACHE[flags] = run
    return run


def kernel(x, Wm0, bm0, U0, V0, b0, Wm1, bm1, U1, V1, b1,
           Wm2, bm2, U2, V2, b2):
    layers = [(Wm0, bm0, U0, V0, b0), (Wm1, bm1, U1, V1, b1),
              (Wm2, bm2, U2, V2, b2)]
    flags = (
        any(np.any(np.asarray(t[1], np.float32)) for t in layers),
        any(np.any(np.asarray(t[4], np.float32)) for t in layers[:2]),
        bool(np.any(np.asarray(layers[2][4], np.float32))),
    )
    run = _get_runner(flags)
    in_maps = _make_in_maps(x, layers, flags, reps=1)
    try:
        res = run(in_maps)
    except Exception:
        # transient NRT execution errors have been observed on this fabric;
        # one retry on fresh device buffers
        _RUNNER_CACHE.pop(flags, None)
        run = _get_runner(flags)
        res = run(in_maps)
    # res["out"] is the concatenation of the 8 per-core [BL, D] shards
    return np.ascontiguousarray(res["out"]).astype(np.float32)



# revision 39
# speedup vs baseline: 1.0071x; 1.0071x over previous
"""Trainium2 Bass kernel for nn_APG_MLP_Layer (3-layer APG hyper-network MLP).

Reference computation per layer (B=8192, din=dout=1024, RANK=64):
    w = (x @ Wm.T + bm).reshape(B, 64, 64)   # per-sample generated weights
    u = x @ U.T                              # [B, 64]
    h = einsum('br,brs->bs', u, w)           # per-sample vec-mat product
    out = relu?(h @ V.T + b)

Sharding: data-parallel over batch across 8 NeuronCores (1024 rows/core);
static params replicated.

Device mapping (per core, per 128-row batch tile):
  - Wm GEMM dominates (8192x1024x4096 per layer). Host pre-transposes all
    static operands and reorders Wm rows to j' = s*64 + r so that each PSUM
    chunk [128b, 512] holds w'[b, s_block(8), r(64)] with r contiguous.
  - The einsum contraction is then one DVE tensor_tensor multiply with u
    broadcast over s (step-0 AP) + one inner-axis tensor_reduce -> h[b, s].
  - h (bf16) is PE-transposed in PAIRS of batch tiles: one [128,128]
    transpose covers two bts and lands ht(bt0) on partitions 0..63 /
    ht(bt1) on 64..127, so the V GEMMs run pairwise-concurrent in disjoint
    PE row groups (vt2 = V.T duplicated on both partition halves). Layers
    0/1 compute outT[o, b] (ReLU'd output directly the next layer's lhsT);
    layer 2 computes out[b, o]. The pair tail is emitted one j-chunk into
    the next bt's wm stream so the DVE lag never stalls the PE.
  - All matmuls run in bf16 (fp32 accumulate in PSUM).

The kernel has a runtime `reps` loop (register-bound For_i) so the same NEFF
serves correctness (reps=1) and steady-state timing (reps=R, slope method).
"""

import numpy as np
import ml_dtypes

import concourse.bass as bass
import concourse.mybir as mybir
from concourse import bacc
from concourse.tile import TileContext
from concourse.masks import make_identity

BF16 = ml_dtypes.bfloat16
FP32 = mybir.dt.float32
BF = mybir.dt.bfloat16

B = 8192
NCORES = 8
BL = B // NCORES          # 1024 rows per core
D = 1024                  # all layer dims
R = 64                    # rank
NBT = BL // 128           # batch tiles per core (8)
NK = D // 128             # k chunks (8)
NJ = (R * R) // 512       # j chunks of 512 (8)
NOC = D // 128            # output chunks (8)


def _ldw_key(inst):
    a = inst.ins[0]
    if getattr(a, "dynamic_ap_info", None) is not None:
        return None
    return (a.memref, a.offset, str(a.ap), str(a.dtype),
            inst.tile_position, inst.tile_size, str(inst.perf_mode),
            inst.is_transpose)


def dedup_ldweights(nc):
    """Post-compile pass: drop InstLdweights that reload the exact weights
    already resident in the PE array (same AP, no intervening clobber).
    The lowered InstMatmults are non-self-loading (ldweights=False), so a
    dropped redundant load is semantics-preserving. LDWs carrying semaphore
    waits are kept."""
    removed = 0
    for f in nc.m.functions:
        for bb in f.blocks:
            insts = list(bb.instructions)
            cur = None
            out = []
            changed = False
            for inst in insts:
                tn = type(inst).__name__
                if str(inst.engine) != "EngineType.PE":
                    out.append(inst)
                    continue
                if tn == "InstLdweights":
                    key = _ldw_key(inst)
                    if key is not None and key == cur and not inst.has_wait():
                        removed += 1
                        changed = True
                        continue
                    cur = key
                    out.append(inst)
                elif tn in ("InstMatmult", "InstMatmultMx"):
                    if getattr(inst, "is_transpose", None):
                        cur = None
                    out.append(inst)
                else:
                    cur = None
                    out.append(inst)
            if changed:
                bb.instructions = out
    return removed


def batch_mm_updates(nc, every=8):
    """Timing experiment: batch per-MM semaphore increments — keep one inc of
    value `every` on each every-th MM, drop the rest. Only valid when nothing
    waits on intermediate values of the PE semaphore (pe_wm ablations)."""
    import concourse.mybir as mybir
    nbat = 0
    for f in nc.m.functions:
        for bb in f.blocks:
            insts = list(bb.instructions)
            mms = [i for i in insts
                   if type(i).__name__ == "InstMatmult"
                   and str(i.engine) == "EngineType.PE"
                   and i.sync_info is not None
                   and len(i.sync_info.on_update) == 1
                   and not i.sync_info.on_wait]
            if len(mms) < every:
                continue
            # group by target semaphore id
            from collections import defaultdict
            by_sem = defaultdict(list)
            for i in mms:
                u = i.sync_info.on_update[0]
                if u.update_mode == "sem-inc" and u.update_value == 1:
                    by_sem[u.id].append(i)
            for sem, lst in by_sem.items():
                n = len(lst)
                nfull = n // every
                for idx, inst in enumerate(lst):
                    gi = idx // every
                    if gi >= nfull:
                        continue  # leave the remainder with inc 1
                    si = inst.sync_info
                    if (idx + 1) % every == 0:
                        u = si.on_update[0]
                        u.update_value = every
                        inst.sync_info = si
                        nbat += 1
                    else:
                        si.on_update = []
                        inst.sync_info = si
    return nbat


def build_apg(include_bm=False, include_b01=False, include_b2=False,
              reps_loop=True, pipeline=False, u_fold=True, v_dma_t=False,
              h_dma_t=False, wm_gp=False, v_batch=False, ablate=None,
              loop_kwargs=None, tmp_bufs=4, wm_bufs=16, act_bufs=3,
              wm_n=512, pw_bufs=None, persist=(0, 0, 0), persist_xt=False,
              osb_bufs=2, v_pack=False, po_bufs=None, dedup_ldw=False,
              batch_updates=0, xt_split=False, wm_split=1, v_pair=True,
              h_bf=True, pair_defer=True, tmp_bf=False):
    """Build + compile the Bass module. Returns (nc, names) where names lists
    the DRAM input tensor names in declaration order."""
    import contextlib
    wm_nodma = ablate == "pe_wm_nodma"
    if wm_nodma:
        ablate = "pe_wm"
    ko = ablate == "pe_wm_ko"
    if ko:
        ablate = "pe_wm"
    if v_pack:
        v_batch = True
    nc = bacc.Bacc("TRN2", target_bir_lowering=False, debug=False,
                   num_devices=NCORES)

    xt = nc.dram_tensor("xt", [D, BL], BF, kind="ExternalInput")
    wmt = [nc.dram_tensor(f"wmt{l}", [D, R * R], BF, kind="ExternalInput")
           for l in range(3)]
    ut = [nc.dram_tensor(f"ut{l}", [D, R], BF, kind="ExternalInput")
          for l in range(3)]
    vt = [nc.dram_tensor(f"vt{l}", [R, D], BF, kind="ExternalInput")
          for l in range(3)]
    bm_row = b01_col = b2_row = None
    if include_bm:
        bm_row = [nc.dram_tensor(f"bmr{l}", [1, R * R], BF, kind="ExternalInput")
                  for l in range(3)]
    if include_b01:
        b01_col = [nc.dram_tensor(f"b{l}c", [128, NOC], FP32, kind="ExternalInput")
                   for l in range(2)]
    if include_b2:
        b2_row = nc.dram_tensor("b2r", [1, D], BF, kind="ExternalInput")
    reps_t = None
    if reps_loop:
        reps_t = nc.dram_tensor("reps", [1, 1], mybir.dt.uint32,
                                kind="ExternalInput")
    out_d = nc.dram_tensor("out", [BL, D], FP32, kind="ExternalOutput")

    with TileContext(nc) as tc:
        with (
            tc.tile_pool(name="const", bufs=1) as constp,
            tc.tile_pool(name="xt", bufs=2) as xtp,
            tc.tile_pool(name="wm", bufs=wm_bufs) as wmp,
            tc.tile_pool(name="usb", bufs=act_bufs) as usbp,
            tc.tile_pool(name="h", bufs=act_bufs) as hp,
            tc.tile_pool(name="ht", bufs=act_bufs) as htp,
            tc.tile_pool(name="tmp", bufs=tmp_bufs) as tmpp,
            tc.tile_pool(name="osb", bufs=osb_bufs) as osbp,
            tc.tile_pool(name="pw", bufs=(pw_bufs if pw_bufs is not None
                                          else 2 if v_pair
                                          else (3 if v_pack else 4)
                                          if wm_n == 512 else 2),
                         space="PSUM") as pwp,
            tc.tile_pool(name="pu", bufs=1, space="PSUM") as pup,
            tc.tile_pool(name="pt", bufs=1, space="PSUM") as ptp,
            tc.tile_pool(name="po", bufs=(po_bufs if po_bufs is not None
                                          else 4 if v_pair
                                          else 3 if v_pack
                                          else 2 if v_batch else 1),
                         space="PSUM") as pop,
            tc.tile_pool(name="ht4", bufs=2) as ht4p,
        ):
            # ---- constants (loaded once, outside the reps loop) ----
            ident = constp.tile([128, 128], FP32, tag="ident")
            make_identity(nc, ident[:, :])
            ident_bf = None
            if h_bf:
                ident_bf = constp.tile([128, 128], BF, tag="identbf")
                make_identity(nc, ident_bf[:, :])
            vt_sb = []
            vt2_sb = []
            for l in range(3):
                if v_pack or v_pair:
                    # V.T duplicated on both partition halves so K=64 V-GEMMs
                    # can run pairwise in disjoint PE row groups
                    t2 = constp.tile([128, D], BF, tag=f"vt2_{l}")
                    nc.sync.dma_start(out=t2[0:R, :], in_=vt[l][:, :])
                    nc.sync.dma_start(out=t2[R:128, :], in_=vt[l][:, :])
                    vt2_sb.append(t2)
                    vt_sb.append(t2)
                else:
                    t = constp.tile([R, D], BF, tag=f"vt{l}")
                    nc.sync.dma_start(out=t[:, :], in_=vt[l][:, :])
                    vt_sb.append(t)
                    vt2_sb.append(None)
            ut_sb = []
            for l in range(3):
                # [128, NK*R]: column block k holds U_l.T rows k*128..k*128+127
                t = constp.tile([128, NK * R], BF, tag=f"ut{l}")
                nc.sync.dma_start(
                    out=t[:, :].rearrange("p (k r) -> p k r", r=R),
                    in_=ut[l][:, :].rearrange("(k p) r -> p k r", p=128))
                ut_sb.append(t)
            ones_bf = None
            if include_bm or include_b2:
                ones_bf = constp.tile([1, 128], BF, tag="ones")
                nc.vector.memset(ones_bf[:, :], 1.0)
            bmr_sb = []
            if include_bm:
                for l in range(3):
                    t = constp.tile([1, R * R], BF, tag=f"bmr{l}")
                    nc.sync.dma_start(out=t[:, :], in_=bm_row[l][:, :])
                    bmr_sb.append(t)
            b01_sb = []
            if include_b01:
                for l in range(2):
                    t = constp.tile([128, NOC], FP32, tag=f"b01_{l}")
                    nc.sync.dma_start(out=t[:, :], in_=b01_col[l][:, :])
                    b01_sb.append(t)
            b2_sb = None
            if include_b2:
                b2_sb = constp.tile([1, D], BF, tag="b2")
                nc.sync.dma_start(out=b2_sb[:, :], in_=b2_row[:, :])

            # runtime rep count on all engines
            if reps_loop:
                regs = nc.alloc_registers("reps_regs", mybir.ALL_ENGINES)
                nc.regs_load(regs, reps_t[0:1, 0:1])
                reps_val = nc.snap(regs, donate=True, min_val=1, max_val=1 << 20)
                loop_cm = tc.For_i(0, reps_val, 1, **(loop_kwargs or {}))
            else:
                loop_cm = contextlib.nullcontext()

            wm_static = None
            if wm_nodma:
                # one wm tile set loaded outside the reps loop, reused for
                # all layers (timing ablation only — results are wrong)
                wm_static = []
                for k in range(NK):
                    t = constp.tile([128, R * R], BF, tag=f"wmstat{k}")
                    nc.sync.dma_start(out=t[:, :],
                                      in_=wmt[0][k * 128:(k + 1) * 128, :])
                    wm_static.append(t)

            # weight-stationary: persist the first persist[l] wm tiles of each
            # layer in SBUF (loaded once, outside the reps loop)
            wm_persist = {}
            for l in range(3):
                for k in range(persist[l]):
                    t = constp.tile([128, R * R], BF, tag=f"wmp{l}_{k}")
                    nc.sync.dma_start(out=t[:, :],
                                      in_=wmt[l][k * 128:(k + 1) * 128, :])
                    wm_persist[(l, k)] = t
            xt_static = None
            if persist_xt:
                xt_static = constp.tile([128, NK * BL], BF, tag="xt0")
                nc.sync.dma_start(
                    out=xt_static[:, :].rearrange("p (k b) -> p k b", b=BL),
                    in_=xt[:, :].rearrange("(k p) b -> p k b", p=128))

            with loop_cm:
                # activations (lhsT layout): [128, NK*BL] bf16; col block k
                # holds x.T rows k*128..k*128+127 (i.e. x cols), b along free.
                if persist_xt:
                    xt_cur = xt_static
                else:
                    xt_cur = xtp.tile([128, NK * BL], BF, tag="act")
                    if xt_split:
                        # per-k-chunk DMAs so bt0's first matmuls only wait
                        # on the first 256KB, not the whole 2MB
                        for k in range(NK):
                            nc.sync.dma_start(
                                out=xt_cur[:, k * BL:(k + 1) * BL],
                                in_=xt[k * 128:(k + 1) * 128, :])
                    else:
                        nc.sync.dma_start(
                            out=xt_cur[:, :].rearrange("p (k b) -> p k b", b=BL),
                            in_=xt[:, :].rearrange("(k p) b -> p k b", p=128))

                # Software pipeline over (layer, batch-tile): each
                # iteration's tail (h transpose + V GEMM + relu/output) is
                # emitted interleaved into the NEXT iteration's wm-GEMM
                # stream so its small LDWEIGHTS-bound matmuls hide behind
                # the 512-column wm matmuls. `pending` holds the tail
                # closures of the previous (l, bt).
                pending = []
                pending_pair = [None]

                def emit_slot():
                    if pending:
                        pending.pop(0)()

                def make_pair_tail(l, b0, b1, hpair_t, xt_next):
                    def run():
                        if h_bf:
                            pt = ptp.tile([128, 128], BF, tag="ptp")
                            nc.tensor.transpose(pt[:, :], hpair_t[:, :],
                                                ident_bf[:, :])
                        else:
                            pt = ptp.tile([128, 128], FP32, tag="ptp")
                            nc.tensor.transpose(pt[:, :], hpair_t[:, :],
                                                ident[:, :])
                        ht2 = htp.tile([128, 128], BF, tag="ht2")
                        nc.scalar.copy(ht2[:, :], pt[:, :])
                        vt2 = vt2_sb[l]
                        if l < 2:
                            for half in range(2):
                                oc0 = half * 4
                                poa = pop.tile([128, 512], FP32, tag="po")
                                pob = pop.tile([128, 512], FP32, tag="po")
                                for oc in range(oc0, oc0 + 4):
                                    csl = slice((oc - oc0) * 128,
                                                (oc - oc0 + 1) * 128)
                                    nc.tensor.matmul(
                                        poa[:, csl],
                                        vt2[0:R, oc * 128:(oc + 1) * 128],
                                        ht2[0:R, :],
                                        start=True, stop=True)
                                    nc.tensor.matmul(
                                        pob[:, csl],
                                        vt2[R:128, oc * 128:(oc + 1) * 128],
                                        ht2[R:128, :],
                                        start=True, stop=True)
                                for bx, po_t in ((b0, poa), (b1, pob)):
                                    dst = (xt_next[:, :]
                                           .rearrange("p (k b) -> p k b",
                                                      b=BL)
                                           [:, oc0:oc0 + 4,
                                            bx * 128:bx * 128 + 128])
                                    nc.scalar.activation(
                                        dst,
                                        po_t[:, :].rearrange(
                                            "p (k c) -> p k c", c=128),
                                        mybir.ActivationFunctionType.Relu)
                        else:
                            o_sb0 = osbp.tile([128, D], FP32, tag="osb")
                            o_sb1 = osbp.tile([128, D], FP32, tag="osb")
                            for half in range(2):
                                osl = slice(half * 512, (half + 1) * 512)
                                poa = pop.tile([128, 512], FP32, tag="po")
                                pob = pop.tile([128, 512], FP32, tag="po")
                                nc.tensor.matmul(
                                    poa[:, :], ht2[0:R, :], vt2[0:R, osl],
                                    start=True, stop=True)
                                nc.tensor.matmul(
                                    pob[:, :], ht2[R:128, :],
                                    vt2[R:128, osl],
                                    start=True, stop=True)
                                nc.scalar.copy(o_sb0[:, osl], poa[:, :])
                                nc.vector.tensor_copy(out=o_sb1[:, osl],
                                                      in_=pob[:, :])
                            nc.sync.dma_start(
                                out=out_d[b0 * 128:b0 * 128 + 128, :],
                                in_=o_sb0[:, :])
                            nc.sync.dma_start(
                                out=out_d[b1 * 128:b1 * 128 + 128, :],
                                in_=o_sb1[:, :])
                    return run

                def make_tail(l, bt, h_sb, xt_next, bsl):
                    vt_t = vt_sb[l]
                    items = []

                    if h_dma_t:
                        # keep the transpose off the PE: cast h to bf16 on
                        # DVE, transpose via the DMA xbar. The xbar wants
                        # 128x128 tiles, so pad: only cols 0:64 of h_pad are
                        # written and only rows 0:64 of ht_pad are read.
                        h_pad = hp.tile([128, 128], BF, tag="h_bf")
                        ht_pad = htp.tile([128, 128], BF, tag="ht_pad")
                        ht_sb = ht_pad[0:R, :]

                        def t_transpose():
                            nc.vector.tensor_copy(out=h_pad[:, 0:R], in_=h_sb[:, :])
                            nc.sync.dma_start_transpose(out=ht_pad[:, :],
                                                        in_=h_pad[:, :])
                    else:
                        pt = ptp.tile([R, 128], FP32)
                        ht_tile = htp.tile([R, 128], BF)
                        ht_sb = ht_tile[:, :]

                        def t_transpose():
                            nc.tensor.transpose(pt[:, :], h_sb[:, :], ident[:, :])
                            nc.scalar.copy(ht_sb, pt[:, :])
                    items.append(t_transpose)

                    if l < 2 and v_dma_t:
                        # V GEMM in [b, o] layout with hT stationary (one
                        # LDWEIGHTS), relu to bf16, then DMA-xbar transpose
                        # each [128,128] block into the next layer's lhsT.
                        po = pop.tile([128, D], FP32)
                        for half in range(2):
                            def t_v2a(half=half):
                                osl = slice(half * 512, (half + 1) * 512)
                                nc.tensor.matmul(
                                    po[:, osl], ht_sb[:, :], vt_t[:, osl],
                                    start=True, stop=True)
                            items.append(t_v2a)

                        def t_relu_t():
                            o_bf = osbp.tile([128, D], BF, tag="obf")
                            nc.scalar.activation(
                                o_bf[:, :], po[:, :],
                                mybir.ActivationFunctionType.Relu)
                            for oc in range(NOC):
                                nc.sync.dma_start_transpose(
                                    out=xt_next[:, oc * BL + bt * 128:
                                                oc * BL + bt * 128 + 128],
                                    in_=o_bf[:, oc * 128:(oc + 1) * 128])
                        items.append(t_relu_t)
                    elif l < 2:
                        po = pop.tile([128, NOC * 128], FP32)
                        for oc in range(NOC):
                            def t_v(oc=oc):
                                nc.tensor.matmul(
                                    po[:, oc * 128:(oc + 1) * 128],
                                    vt_t[:, oc * 128:(oc + 1) * 128],
                                    ht_sb[:, :], start=True, stop=True)
                            items.append(t_v)

                        def t_relu():
                            if include_b01:
                                for oc in range(NOC):
                                    nc.scalar.activation(
                                        xt_next[:, oc * BL + bt * 128:
                                                oc * BL + bt * 128 + 128],
                                        po[:, oc * 128:(oc + 1) * 128],
                                        mybir.ActivationFunctionType.Relu,
                                        bias=b01_sb[l][:, oc:oc + 1], scale=1.0)
                            else:
                                nc.scalar.activation(
                                    xt_next[:, :]
                                    .rearrange("p (k b) -> p k b", b=BL)
                                    [:, :, bt * 128:bt * 128 + 128],
                                    po[:, :].rearrange("p (k c) -> p k c", c=128),
                                    mybir.ActivationFunctionType.Relu)
                        items.append(t_relu)
                    else:
                        po = pop.tile([128, D], FP32)
                        for half in range(2):
                            def t_v2(half=half):
                                osl = slice(half * 512, (half + 1) * 512)
                                nc.tensor.matmul(
                                    po[:, osl], ht_sb[:, :], vt_t[:, osl],
                                    start=True, stop=not include_b2)
                                if include_b2:
                                    nc.tensor.matmul(
                                        po[:, osl], ones_bf[:, :], b2_sb[:, osl],
                                        start=False, stop=True)
                            items.append(t_v2)

                        def t_out():
                            o_sb = osbp.tile([128, D], FP32)
                            nc.scalar.copy(o_sb[:, :], po[:, :])
                            nc.sync.dma_start(out=out_d[bsl, :], in_=o_sb[:, :])
                        items.append(t_out)
                    return items

                wm_tiles = None
                xt_l = xt_cur
                for l in range(3):
                    if wm_nodma:
                        wm_tiles = wm_static
                    else:
                        wm_tiles = []
                        dma_eng = nc.gpsimd if wm_gp else nc.sync
                        for k in range(NK):
                            if (l, k) in wm_persist:
                                wm_tiles.append(wm_persist[(l, k)])
                                continue
                            t = wmp.tile([128, R * R], BF, tag="wm")
                            if wm_split > 1:
                                step = (R * R) // wm_split
                                for s0 in range(0, R * R, step):
                                    dma_eng.dma_start(
                                        out=t[:, s0:s0 + step],
                                        in_=wmt[l][k * 128:(k + 1) * 128,
                                                   s0:s0 + step])
                            else:
                                dma_eng.dma_start(
                                    out=t[:, :],
                                    in_=wmt[l][k * 128:(k + 1) * 128, :])
                            wm_tiles.append(t)
                    xt_next = None
                    if l < 2 and ablate != "pe_wm":
                        xt_next = xtp.tile([128, NK * BL], BF, tag="act")

                    hpair = None
                    for bt in range(NBT):
                        bsl = slice(bt * 128, (bt + 1) * 128)
                        xt_b = xt_l

                        def lhs(k, xt_b=xt_b, bt=bt):
                            return xt_b[:, k * BL + bt * 128:
                                        k * BL + bt * 128 + 128]

                        if ko:
                            # k-outer / j-inner: 8 consecutive MMs share the
                            # same stationary lhsT (tests LDW elision)
                            nj = (R * R) // wm_n
                            pws = [pwp.tile([128, wm_n], FP32, tag="pwko",
                                            name=f"pwko{j}")
                                   for j in range(nj)]
                            for k in range(NK):
                                for j in range(nj):
                                    wm_mv = wm_tiles[k][:, j * wm_n:
                                                        (j + 1) * wm_n]
                                    pw_out = pws[j][:, :]
                                    if wm_n > 512:
                                        wm_mv = wm_mv.rearrange(
                                            "p (t n) -> p t n", n=512)
                                        pw_out = pw_out.rearrange(
                                            "p (t n) -> p t n", n=512)
                                    nc.tensor.matmul(
                                        pw_out, lhs(k), wm_mv,
                                        start=(k == 0), stop=(k == NK - 1))
                            if l == 2 and bt == NBT - 1:
                                nc.sync.dma_start(
                                    out=out_d[bsl, :],
                                    in_=xt_b[:, :].bitcast(FP32)[:, 0:D])
                            continue
                        pu = u_sb = h_sb = None
                        if ablate != "pe_wm":
                            pu = pup.tile([128, R], FP32)
                            u_sb = usbp.tile([128, R], FP32)
                            if v_pair:
                                if bt % 2 == 0:
                                    hpair = hp.tile([128, 2 * R],
                                                    BF if h_bf else FP32,
                                                    tag="hpair")
                                # single-step slicing off the tile (NOT a
                                # view-of-a-view) so dependency ranges are
                                # exact
                                h_sb = hpair
                                h_base = (bt % 2) * R
                            else:
                                h_sb = hp.tile([128, R], FP32)
                                h_base = 0
                        if not u_fold:
                            for k in range(NK):
                                nc.tensor.matmul(pu[:, :], lhs(k),
                                                 ut_sb[l][:, k * R:(k + 1) * R],
                                                 start=(k == 0),
                                                 stop=(k == NK - 1))
                            nc.scalar.copy(u_sb[:, :], pu[:, :])
                        nj = (R * R) // wm_n
                        s_per = wm_n // R
                        for j in range(nj):
                            if j == 1 and pending_pair[0] is not None:
                                # deferred pair tail rides here, one chunk
                                # into the next bt's stream, so the DVE lag
                                # on the pair's last chunk never stalls PE
                                pending_pair[0]()
                                pending_pair[0] = None
                            pw = pwp.tile([128, wm_n], FP32)
                            for k in range(NK):
                                wm_mv = wm_tiles[k][:, j * wm_n:(j + 1) * wm_n]
                                pw_out = pw[:, :]
                                if wm_n > 512:
                                    # ISA caps a single AP dim at 512 elements
                                    wm_mv = wm_mv.rearrange(
                                        "p (t n) -> p t n", n=512)
                                    pw_out = pw_out.rearrange(
                                        "p (t n) -> p t n", n=512)
                                nc.tensor.matmul(
                                    pw_out, lhs(k), wm_mv,
                                    start=(k == 0),
                                    stop=(k == NK - 1) and not include_bm)
                                if j == 0 and u_fold and ablate != "pe_wm":
                                    # u GEMM rides the j=0 chunk so its
                                    # LDWEIGHTS hide behind wide matmuls
                                    nc.tensor.matmul(
                                        pu[:, :], lhs(k),
                                        ut_sb[l][:, k * R:(k + 1) * R],
                                        start=(k == 0), stop=(k == NK - 1))
                            if include_bm:
                                nc.tensor.matmul(
                                    pw[:, :], ones_bf[:, :],
                                    bmr_sb[l][:, j * wm_n:(j + 1) * wm_n],
                                    start=False, stop=True)
                            if j == 0 and u_fold and ablate != "pe_wm":
                                nc.scalar.copy(u_sb[:, :], pu[:, :])
                            if ablate == "nodve":
                                nc.vector.tensor_copy(
                                    out=h_sb[:, h_base + j * s_per:
                                             h_base + (j + 1) * s_per],
                                    in_=pw[:, 0:s_per])
                            if ablate != "nodve" and ablate != "pe_wm":
                                # tmp[b, s, r] = w'[b, s, r] * u[b, r]
                                tmp = tmpp.tile([128, wm_n],
                                                BF if tmp_bf else FP32)
                                nc.vector.tensor_tensor(
                                    out=tmp[:, :].rearrange("p (s r) -> p s r", r=R),
                                    in0=pw[:, :].rearrange("p (s r) -> p s r", r=R),
                                    in1=u_sb[:, :].unsqueeze(1)
                                    .broadcast_to([128, s_per, R]),
                                    op=mybir.AluOpType.mult)
                                # h[b, jc*s_per + s] = sum_r tmp[b, s, r]
                                import contextlib as _ctl
                                _lp = (nc.allow_low_precision(
                                    "h is cast to bf16 before the V GEMM "
                                    "anyway; rounding at the reduce is "
                                    "equivalent") if h_bf
                                    else _ctl.nullcontext())
                                with _lp:
                                    nc.vector.tensor_reduce(
                                        out=h_sb[:, h_base + j * s_per:
                                                 h_base + (j + 1) * s_per],
                                        in_=tmp[:, :].rearrange("p (s r) -> p s r", r=R),
                                        axis=mybir.AxisListType.X,
                                        op=mybir.AluOpType.add)
                            for _ in range(wm_n // 512):
                                emit_slot()
                            if j == 0 or j == nj - 1:
                                emit_slot()
                                emit_slot()
                        assert not pending, "tail did not fit in slot budget"
                        if v_pair and ablate is None:
                            # paired-bt tail: one [128,128] transpose covers
                            # two bts; its output lands ht(bt0) on partitions
                            # 0..63 and ht(bt1) on 64..127, so the V GEMMs run
                            # pairwise-concurrent in disjoint PE row groups.
                            if bt % 2 == 0:
                                continue
                            fn = make_pair_tail(l, bt - 1, bt, hpair, xt_next)
                            if pair_defer:
                                pending_pair[0] = fn
                            else:
                                fn()
                            continue
                        if v_batch and ablate is None:
                            # transpose h into the 4-tile group's shared hT
                            # operand; run the V GEMM once per group at
                            # n=512 so LDWEIGHTS amortizes.
                            if bt % 4 == 0:
                                ht4 = ht4p.tile([128 if v_pack else R, 512],
                                                BF, tag="ht4")
                            pt = ptp.tile([R, 128], FP32)
                            nc.tensor.transpose(pt[:, :], h_sb[:, :], ident[:, :])
                            nc.scalar.copy(
                                ht4[0:R, (bt % 4) * 128:(bt % 4 + 1) * 128],
                                pt[:, :])
                            if v_pack:
                                # second copy on partitions 64..127 feeds the
                                # row-group-packed partner matmul
                                nc.scalar.copy(
                                    ht4[R:128, (bt % 4) * 128:(bt % 4 + 1) * 128],
                                    pt[:, :])
                            if bt % 4 == 3:
                                g0 = (bt - 3) * 128
                                if l < 2 and v_pack:
                                    for ocp in range(NOC // 2):
                                        oc0, oc1 = 2 * ocp, 2 * ocp + 1
                                        po0 = pop.tile([128, 512], FP32, tag="po")
                                        po1 = pop.tile([128, 512], FP32, tag="po")
                                        nc.tensor.matmul(
                                            po0[:, :],
                                            vt2_sb[l][0:R, oc0 * 128:(oc0 + 1) * 128],
                                            ht4[0:R, :], start=True, stop=True)
                                        nc.tensor.matmul(
                                            po1[:, :],
                                            vt2_sb[l][R:128, oc1 * 128:(oc1 + 1) * 128],
                                            ht4[R:128, :], start=True, stop=True)
                                        for oc, po in ((oc0, po0), (oc1, po1)):
                                            dst = xt_next[:, oc * BL + g0:
                                                          oc * BL + g0 + 512]
                                            if include_b01:
                                                nc.scalar.activation(
                                                    dst, po[:, :],
                                                    mybir.ActivationFunctionType.Relu,
                                                    bias=b01_sb[l][:, oc:oc + 1],
                                                    scale=1.0)
                                            elif oc % 2 == 1:
                                                # split relus across ACT and DVE
                                                nc.vector.tensor_scalar_max(
                                                    dst, po[:, :], 0.0)
                                            else:
                                                nc.scalar.activation(
                                                    dst, po[:, :],
                                                    mybir.ActivationFunctionType.Relu)
                                elif l < 2:
                                    for oc in range(NOC):
                                        po = pop.tile([128, 512], FP32, tag="po")
                                        nc.tensor.matmul(
                                            po[:, :],
                                            vt_sb[l][:, oc * 128:(oc + 1) * 128],
                                            ht4[:, :], start=True, stop=True)
                                        if include_b01:
                                            nc.scalar.activation(
                                                xt_next[:, oc * BL + g0:
                                                        oc * BL + g0 + 512],
                                                po[:, :],
                                                mybir.ActivationFunctionType.Relu,
                                                bias=b01_sb[l][:, oc:oc + 1],
                                                scale=1.0)
                                        else:
                                            nc.scalar.activation(
                                                xt_next[:, oc * BL + g0:
                                                        oc * BL + g0 + 512],
                                                po[:, :],
                                                mybir.ActivationFunctionType.Relu)
                                elif v_pack:
                                    assert not include_b2
                                    for i4 in range(4):
                                        o_sb = osbp.tile([128, D], FP32)
                                        po0 = pop.tile([128, 512], FP32, tag="po")
                                        po1 = pop.tile([128, 512], FP32, tag="po")
                                        nc.tensor.matmul(
                                            po0[:, :],
                                            ht4[0:R, i4 * 128:(i4 + 1) * 128],
                                            vt2_sb[l][0:R, 0:512],
                                            start=True, stop=True)
                                        nc.tensor.matmul(
                                            po1[:, :],
                                            ht4[R:128, i4 * 128:(i4 + 1) * 128],
                                            vt2_sb[l][R:128, 512:1024],
                                            start=True, stop=True)
                                        nc.scalar.copy(o_sb[:, 0:512], po0[:, :])
                                        nc.vector.tensor_copy(
                                            out=o_sb[:, 512:1024], in_=po1[:, :])
                                        row0 = (bt - 3 + i4) * 128
                                        nc.sync.dma_start(
                                            out=out_d[row0:row0 + 128, :],
                                            in_=o_sb[:, :])
                                else:
                                    for i4 in range(4):
                                        o_sb = osbp.tile([128, D], FP32)
                                        for half in range(2):
                                            osl = slice(half * 512,
                                                        (half + 1) * 512)
                                            po = pop.tile([128, 512], FP32,
                                                          tag="po")
                                            nc.tensor.matmul(
                                                po[:, :],
                                                ht4[:, i4 * 128:(i4 + 1) * 128],
                                                vt_sb[l][:, osl],
                                                start=True, stop=not include_b2)
                                            if include_b2:
                                                nc.tensor.matmul(
                                                    po[:, :], ones_bf[:, :],
                                                    b2_sb[:, osl],
                                                    start=False, stop=True)
                                            nc.scalar.copy(o_sb[:, osl],
                                                           po[:, :])
                                        row0 = (bt - 3 + i4) * 128
                                        nc.sync.dma_start(
                                            out=out_d[row0:row0 + 128, :],
                                            in_=o_sb[:, :])
                            continue
                        if ablate == "pe_wm":
                            if l == 2 and bt == NBT - 1:
                                # keep the output write so the graph has one
                                nc.sync.dma_start(
                                    out=out_d[bsl, :],
                                    in_=xt_b[:, :].bitcast(FP32)[:, 0:D])
                            continue
                        pending = make_tail(l, bt, h_sb, xt_next, bsl)
                        if not pipeline:
                            for fn in pending:
                                fn()
                            pending = []
                    if xt_next is not None:
                        xt_l = xt_next

                if pending_pair[0] is not None:
                    pending_pair[0]()
                    pending_pair[0] = None
                for fn in pending:
                    fn()
                pending = []
    nc.compile()
    if dedup_ldw:
        n = dedup_ldweights(nc)
        print(f"dedup_ldweights: removed {n}")
    if batch_updates:
        n = batch_mm_updates(nc, every=batch_updates)
        print(f"batch_mm_updates: batched {n}")
    return nc


# ---------------------------------------------------------------------------
# host side
# ---------------------------------------------------------------------------

def _prep_static(Wm, bm, U, V, b):
    """Host-side layout prep of one layer's replicated params."""
    Wm = np.asarray(Wm, dtype=np.float32)
    # rows j' = s*64 + r  <->  original j = r*64 + s ; then transpose -> [k, j']
    wmt = np.ascontiguousarray(
        Wm.reshape(R, R, D).transpose(1, 0, 2).reshape(R * R, D).T).astype(BF16)
    utm = np.ascontiguousarray(np.asarray(U, dtype=np.float32).T).astype(BF16)
    vtm = np.ascontiguousarray(np.asarray(V, dtype=np.float32).T).astype(BF16)
    return wmt, utm, vtm


_CACHE = {}


def _get_compiled(flags):
    if flags not in _CACHE:
        _CACHE[flags] = build_apg(*flags)
    return _CACHE[flags]


def _make_in_maps(x, layers, flags, reps):
    include_bm, include_b01, include_b2 = flags
    x = np.asarray(x, dtype=np.float32)
    shared = {}
    for l, (Wm, bm, U, V, b) in enumerate(layers):
        wmt, utm, vtm = _prep_static(Wm, bm, U, V, b)
        shared[f"wmt{l}"] = wmt
        shared[f"ut{l}"] = utm
        shared[f"vt{l}"] = vtm
        if include_bm:
            shared[f"bmr{l}"] = np.asarray(bm, np.float32).reshape(R, R).T \
                .reshape(1, R * R).astype(BF16)
        if include_b01 and l < 2:
            shared[f"b{l}c"] = np.ascontiguousarray(
                np.asarray(b, np.float32).reshape(NOC, 128).T)
        if include_b2 and l == 2:
            shared["b2r"] = np.asarray(b, np.float32).reshape(1, D).astype(BF16)
    shared["reps"] = np.array([[reps]], dtype=np.uint32)
    in_maps = []
    for i in range(NCORES):
        m = dict(shared)
        xs = x[i * BL:(i + 1) * BL, :]
        m["xt"] = np.ascontiguousarray(xs.T).astype(BF16)
        in_maps.append(m)
    return in_maps


_RUNNER_CACHE = {}


def _get_runner(flags):
    """Jit-once PJRT runner for the compiled module (same execution path as
    bass_utils.run_bass_kernel_spmd's axon redirect through bass2jax, but
    cached so repeat kernel() calls skip re-trace/re-compile)."""
    if flags in _RUNNER_CACHE:
        return _RUNNER_CACHE[flags]
    import jax
    from jax.sharding import Mesh, PartitionSpec, NamedSharding
    from jax.experimental.shard_map import shard_map
    from concourse import bass2jax

    nc = _get_compiled(flags)
    bass2jax.install_neuronx_cc_hook()
    partition_name = nc.partition_id_tensor.name if nc.partition_id_tensor else None
    in_names, out_names, out_avals, zero_outs = [], [], [], []
    for alloc in nc.m.functions[0].allocations:
        if not isinstance(alloc, mybir.MemoryLocationSet):
            continue
        name = alloc.memorylocations[0].name
        if alloc.kind == "ExternalInput":
            if name != partition_name:
                in_names.append(name)
        elif alloc.kind == "ExternalOutput":
            out_names.append(name)
            shape = tuple(alloc.tensor_shape)
            dtype = mybir.dt.np(alloc.dtype)
            out_avals.append(jax.core.ShapedArray(shape, dtype))
            zero_outs.append(np.zeros(shape, dtype))
    n_params = len(in_names)
    all_in_names = list(in_names) + list(out_names)
    if partition_name is not None:
        all_in_names.append(partition_name)

    def _body(*args):
        operands = list(args)
        if partition_name is not None:
            operands = operands + [bass2jax.partition_id_tensor()]
        outs = bass2jax._bass_exec_p.bind(
            *operands, out_avals=tuple(out_avals), in_names=tuple(all_in_names),
            out_names=tuple(out_names), lowering_input_output_aliases=(),
            sim_require_finite=True, sim_require_nnan=True, nc=nc)
        return tuple(outs)

    devices = jax.devices()[:NCORES]
    mesh = Mesh(np.asarray(devices), ("core",))
    in_specs = (PartitionSpec("core"),) * (n_params + len(out_names))
    out_specs = (PartitionSpec("core"),) * len(out_names)
    fn = jax.jit(shard_map(_body, mesh=mesh, in_specs=in_specs,
                           out_specs=out_specs, check_rep=False))
    sh = NamedSharding(mesh, PartitionSpec("core"))

    # Outputs are constant zero-filled donor buffers — upload once.
    zero_dev = [jax.device_put(np.concatenate([z] * NCORES, axis=0), sh)
                for z in zero_outs]
    dev_cache = {}

    def _put_cached(name, arr):
        import hashlib
        key = (name, arr.shape, arr.dtype.str,
               hashlib.blake2b(np.ascontiguousarray(arr).tobytes(),
                               digest_size=16).hexdigest())
        if key not in dev_cache:
            if len(dev_cache) > 64:
                dev_cache.clear()
            dev_cache[key] = jax.device_put(arr, sh)
        return dev_cache[key]

    def run(in_maps):
        dev = [_put_cached(name,
                           np.concatenate([np.asarray(m[name]) for m in in_maps],
                                          axis=0))
               for name in in_names]
        outs = fn(*(dev + zero_dev))
        jax.block_until_ready(outs)
        return {name: np.asarray(outs[i]) for i, name in enumerate(out_names)}

    _RUNNER_CACHE[flags] = run
    return run


def kernel(x, Wm0, bm0, U0, V0, b0, Wm1, bm1, U1, V1, b1,
           Wm2, bm2, U2, V2, b2):
    layers = [(Wm0, bm0, U0, V0, b0), (Wm1, bm1, U1, V1, b1),
              (Wm2, bm2, U2, V2, b2)]
    flags = (
        any(np.any(np.asarray(t[1], np.float32)) for t in layers),
        any(np.any(np.asarray(t[4], np.float32)) for t in layers[:2]),
        bool(np.any(np.asarray(layers[2][4], np.float32))),
    )
    run = _get_runner(flags)
    in_maps = _make_in_maps(x, layers, flags, reps=1)
    try:
        res = run(in_maps)
    except Exception:
        # transient NRT execution errors have been observed on this fabric;
        # one retry on fresh device buffers
        _RUNNER_CACHE.pop(flags, None)
        run = _get_runner(flags)
        res = run(in_maps)
    # res["out"] is the concatenation of the 8 per-core [BL, D] shards
    return np.ascontiguousarray(res["out"]).astype(np.float32)



# revision 41
# speedup vs baseline: 1.0139x; 1.0067x over previous
"""Trainium2 Bass kernel for nn_APG_MLP_Layer (3-layer APG hyper-network MLP).

Reference computation per layer (B=8192, din=dout=1024, RANK=64):
    w = (x @ Wm.T + bm).reshape(B, 64, 64)   # per-sample generated weights
    u = x @ U.T                              # [B, 64]
    h = einsum('br,brs->bs', u, w)           # per-sample vec-mat product
    out = relu?(h @ V.T + b)

Sharding: data-parallel over batch across 8 NeuronCores (1024 rows/core);
static params replicated.

Device mapping (per core, per 128-row batch tile):
  - Wm GEMM dominates (8192x1024x4096 per layer). Host pre-transposes all
    static operands and reorders Wm rows to j' = s*64 + r so that each PSUM
    chunk [128b, 512] holds w'[b, s_block(8), r(64)] with r contiguous.
  - The einsum contraction is then one DVE tensor_tensor multiply with u
    broadcast over s (step-0 AP) + one inner-axis tensor_reduce -> h[b, s].
  - h (bf16) is PE-transposed in PAIRS of batch tiles: one [128,128]
    transpose covers two bts and lands ht(bt0) on partitions 0..63 /
    ht(bt1) on 64..127, so the V GEMMs run pairwise-concurrent in disjoint
    PE row groups (vt2 = V.T duplicated on both partition halves). Layers
    0/1 compute outT[o, b] (ReLU'd output directly the next layer's lhsT);
    layer 2 computes out[b, o]. The pair tail is emitted one j-chunk into
    the next bt's wm stream so the DVE lag never stalls the PE.
  - All matmuls run in bf16 (fp32 accumulate in PSUM).

The kernel has a runtime `reps` loop (register-bound For_i) so the same NEFF
serves correctness (reps=1) and steady-state timing (reps=R, slope method).
"""

import numpy as np
import ml_dtypes

import concourse.bass as bass
import concourse.mybir as mybir
from concourse import bacc
from concourse.tile import TileContext
from concourse.masks import make_identity

BF16 = ml_dtypes.bfloat16
FP32 = mybir.dt.float32
BF = mybir.dt.bfloat16

B = 8192
NCORES = 8
BL = B // NCORES          # 1024 rows per core
D = 1024                  # all layer dims
R = 64                    # rank
NBT = BL // 128           # batch tiles per core (8)
NK = D // 128             # k chunks (8)
NJ = (R * R) // 512       # j chunks of 512 (8)
NOC = D // 128            # output chunks (8)


def _ldw_key(inst):
    a = inst.ins[0]
    if getattr(a, "dynamic_ap_info", None) is not None:
        return None
    return (a.memref, a.offset, str(a.ap), str(a.dtype),
            inst.tile_position, inst.tile_size, str(inst.perf_mode),
            inst.is_transpose)


def dedup_ldweights(nc):
    """Post-compile pass: drop InstLdweights that reload the exact weights
    already resident in the PE array (same AP, no intervening clobber).
    The lowered InstMatmults are non-self-loading (ldweights=False), so a
    dropped redundant load is semantics-preserving. LDWs carrying semaphore
    waits are kept."""
    removed = 0
    for f in nc.m.functions:
        for bb in f.blocks:
            insts = list(bb.instructions)
            cur = None
            out = []
            changed = False
            for inst in insts:
                tn = type(inst).__name__
                if str(inst.engine) != "EngineType.PE":
                    out.append(inst)
                    continue
                if tn == "InstLdweights":
                    key = _ldw_key(inst)
                    if key is not None and key == cur and not inst.has_wait():
                        removed += 1
                        changed = True
                        continue
                    cur = key
                    out.append(inst)
                elif tn in ("InstMatmult", "InstMatmultMx"):
                    if getattr(inst, "is_transpose", None):
                        cur = None
                    out.append(inst)
                else:
                    cur = None
                    out.append(inst)
            if changed:
                bb.instructions = out
    return removed


def batch_mm_updates(nc, every=8):
    """Timing experiment: batch per-MM semaphore increments — keep one inc of
    value `every` on each every-th MM, drop the rest. Only valid when nothing
    waits on intermediate values of the PE semaphore (pe_wm ablations)."""
    import concourse.mybir as mybir
    nbat = 0
    for f in nc.m.functions:
        for bb in f.blocks:
            insts = list(bb.instructions)
            mms = [i for i in insts
                   if type(i).__name__ == "InstMatmult"
                   and str(i.engine) == "EngineType.PE"
                   and i.sync_info is not None
                   and len(i.sync_info.on_update) == 1
                   and not i.sync_info.on_wait]
            if len(mms) < every:
                continue
            # group by target semaphore id
            from collections import defaultdict
            by_sem = defaultdict(list)
            for i in mms:
                u = i.sync_info.on_update[0]
                if u.update_mode == "sem-inc" and u.update_value == 1:
                    by_sem[u.id].append(i)
            for sem, lst in by_sem.items():
                n = len(lst)
                nfull = n // every
                for idx, inst in enumerate(lst):
                    gi = idx // every
                    if gi >= nfull:
                        continue  # leave the remainder with inc 1
                    si = inst.sync_info
                    if (idx + 1) % every == 0:
                        u = si.on_update[0]
                        u.update_value = every
                        inst.sync_info = si
                        nbat += 1
                    else:
                        si.on_update = []
                        inst.sync_info = si
    return nbat


def build_apg(include_bm=False, include_b01=False, include_b2=False,
              reps_loop=True, pipeline=False, u_fold=True, v_dma_t=False,
              h_dma_t=False, wm_gp=False, v_batch=False, ablate=None,
              loop_kwargs=None, tmp_bufs=4, wm_bufs=16, act_bufs=3,
              wm_n=512, pw_bufs=None, persist=(0, 0, 0), persist_xt=False,
              osb_bufs=2, v_pack=False, po_bufs=None, dedup_ldw=False,
              batch_updates=0, xt_split=False, wm_split=1, v_pair=True,
              h_bf=True, pair_defer=True, tmp_bf=False):
    """Build + compile the Bass module. Returns (nc, names) where names lists
    the DRAM input tensor names in declaration order."""
    import contextlib
    wm_nodma = ablate == "pe_wm_nodma"
    if wm_nodma:
        ablate = "pe_wm"
    ko = ablate == "pe_wm_ko"
    if ko:
        ablate = "pe_wm"
    if v_pack:
        v_batch = True
    if include_b01 or include_b2:
        # the paired tail doesn't apply the output biases; use the general
        # per-bt tail (which does) when they are present
        v_pair = False
    nc = bacc.Bacc("TRN2", target_bir_lowering=False, debug=False,
                   num_devices=NCORES)

    xt = nc.dram_tensor("xt", [D, BL], BF, kind="ExternalInput")
    wmt = [nc.dram_tensor(f"wmt{l}", [D, R * R], BF, kind="ExternalInput")
           for l in range(3)]
    ut = [nc.dram_tensor(f"ut{l}", [D, R], BF, kind="ExternalInput")
          for l in range(3)]
    vt = [nc.dram_tensor(f"vt{l}", [R, D], BF, kind="ExternalInput")
          for l in range(3)]
    bm_row = b01_col = b2_row = None
    if include_bm:
        bm_row = [nc.dram_tensor(f"bmr{l}", [1, R * R], BF, kind="ExternalInput")
                  for l in range(3)]
    if include_b01:
        b01_col = [nc.dram_tensor(f"b{l}c", [128, NOC], FP32, kind="ExternalInput")
                   for l in range(2)]
    if include_b2:
        b2_row = nc.dram_tensor("b2r", [1, D], BF, kind="ExternalInput")
    reps_t = None
    if reps_loop:
        reps_t = nc.dram_tensor("reps", [1, 1], mybir.dt.uint32,
                                kind="ExternalInput")
    out_d = nc.dram_tensor("out", [BL, D], FP32, kind="ExternalOutput")

    with TileContext(nc) as tc:
        with (
            tc.tile_pool(name="const", bufs=1) as constp,
            tc.tile_pool(name="xt", bufs=2) as xtp,
            tc.tile_pool(name="wm", bufs=wm_bufs) as wmp,
            tc.tile_pool(name="usb", bufs=act_bufs) as usbp,
            tc.tile_pool(name="h", bufs=act_bufs) as hp,
            tc.tile_pool(name="ht", bufs=act_bufs) as htp,
            tc.tile_pool(name="tmp", bufs=tmp_bufs) as tmpp,
            tc.tile_pool(name="osb", bufs=osb_bufs) as osbp,
            tc.tile_pool(name="pw", bufs=(pw_bufs if pw_bufs is not None
                                          else 2 if v_pair
                                          else (3 if v_pack else 4)
                                          if wm_n == 512 else 2),
                         space="PSUM") as pwp,
            tc.tile_pool(name="pu", bufs=1, space="PSUM") as pup,
            tc.tile_pool(name="pt", bufs=1, space="PSUM") as ptp,
            tc.tile_pool(name="po", bufs=(po_bufs if po_bufs is not None
                                          else 4 if v_pair
                                          else 3 if v_pack
                                          else 2 if v_batch else 1),
                         space="PSUM") as pop,
            tc.tile_pool(name="ht4", bufs=2) as ht4p,
        ):
            # ---- constants (loaded once, outside the reps loop) ----
            ident = constp.tile([128, 128], FP32, tag="ident")
            make_identity(nc, ident[:, :])
            ident_bf = None
            if h_bf:
                ident_bf = constp.tile([128, 128], BF, tag="identbf")
                make_identity(nc, ident_bf[:, :])
            vt_sb = []
            vt2_sb = []
            for l in range(3):
                if v_pack or v_pair:
                    # V.T duplicated on both partition halves so K=64 V-GEMMs
                    # can run pairwise in disjoint PE row groups
                    t2 = constp.tile([128, D], BF, tag=f"vt2_{l}")
                    nc.sync.dma_start(out=t2[0:R, :], in_=vt[l][:, :])
                    nc.sync.dma_start(out=t2[R:128, :], in_=vt[l][:, :])
                    vt2_sb.append(t2)
                    vt_sb.append(t2)
                else:
                    t = constp.tile([R, D], BF, tag=f"vt{l}")
                    nc.sync.dma_start(out=t[:, :], in_=vt[l][:, :])
                    vt_sb.append(t)
                    vt2_sb.append(None)
            ut_sb = []
            for l in range(3):
                # [128, NK*R]: column block k holds U_l.T rows k*128..k*128+127
                t = constp.tile([128, NK * R], BF, tag=f"ut{l}")
                nc.sync.dma_start(
                    out=t[:, :].rearrange("p (k r) -> p k r", r=R),
                    in_=ut[l][:, :].rearrange("(k p) r -> p k r", p=128))
                ut_sb.append(t)
            ones_bf = None
            if include_bm or include_b2:
                ones_bf = constp.tile([1, 128], BF, tag="ones")
                nc.vector.memset(ones_bf[:, :], 1.0)
            bmr_sb = []
            if include_bm:
                for l in range(3):
                    t = constp.tile([1, R * R], BF, tag=f"bmr{l}")
                    nc.sync.dma_start(out=t[:, :], in_=bm_row[l][:, :])
                    bmr_sb.append(t)
            b01_sb = []
            if include_b01:
                for l in range(2):
                    t = constp.tile([128, NOC], FP32, tag=f"b01_{l}")
                    nc.sync.dma_start(out=t[:, :], in_=b01_col[l][:, :])
                    b01_sb.append(t)
            b2_sb = None
            if include_b2:
                b2_sb = constp.tile([1, D], BF, tag="b2")
                nc.sync.dma_start(out=b2_sb[:, :], in_=b2_row[:, :])

            # runtime rep count on all engines
            if reps_loop:
                regs = nc.alloc_registers("reps_regs", mybir.ALL_ENGINES)
                nc.regs_load(regs, reps_t[0:1, 0:1])
                reps_val = nc.snap(regs, donate=True, min_val=1, max_val=1 << 20)
                loop_cm = tc.For_i(0, reps_val, 1, **(loop_kwargs or {}))
            else:
                loop_cm = contextlib.nullcontext()

            wm_static = None
            if wm_nodma:
                # one wm tile set loaded outside the reps loop, reused for
                # all layers (timing ablation only — results are wrong)
                wm_static = []
                for k in range(NK):
                    t = constp.tile([128, R * R], BF, tag=f"wmstat{k}")
                    nc.sync.dma_start(out=t[:, :],
                                      in_=wmt[0][k * 128:(k + 1) * 128, :])
                    wm_static.append(t)

            # weight-stationary: persist the first persist[l] wm tiles of each
            # layer in SBUF (loaded once, outside the reps loop)
            wm_persist = {}
            for l in range(3):
                for k in range(persist[l]):
                    t = constp.tile([128, R * R], BF, tag=f"wmp{l}_{k}")
                    nc.sync.dma_start(out=t[:, :],
                                      in_=wmt[l][k * 128:(k + 1) * 128, :])
                    wm_persist[(l, k)] = t
            xt_static = None
            if persist_xt:
                xt_static = constp.tile([128, NK * BL], BF, tag="xt0")
                nc.sync.dma_start(
                    out=xt_static[:, :].rearrange("p (k b) -> p k b", b=BL),
                    in_=xt[:, :].rearrange("(k p) b -> p k b", p=128))

            with loop_cm:
                # activations (lhsT layout): [128, NK*BL] bf16; col block k
                # holds x.T rows k*128..k*128+127 (i.e. x cols), b along free.
                if persist_xt:
                    xt_cur = xt_static
                else:
                    xt_cur = xtp.tile([128, NK * BL], BF, tag="act")
                    if xt_split:
                        # per-k-chunk DMAs so bt0's first matmuls only wait
                        # on the first 256KB, not the whole 2MB
                        for k in range(NK):
                            nc.sync.dma_start(
                                out=xt_cur[:, k * BL:(k + 1) * BL],
                                in_=xt[k * 128:(k + 1) * 128, :])
                    else:
                        nc.sync.dma_start(
                            out=xt_cur[:, :].rearrange("p (k b) -> p k b", b=BL),
                            in_=xt[:, :].rearrange("(k p) b -> p k b", p=128))

                # Software pipeline over (layer, batch-tile): each
                # iteration's tail (h transpose + V GEMM + relu/output) is
                # emitted interleaved into the NEXT iteration's wm-GEMM
                # stream so its small LDWEIGHTS-bound matmuls hide behind
                # the 512-column wm matmuls. `pending` holds the tail
                # closures of the previous (l, bt).
                pending = []
                pending_pair = [None]

                def emit_slot():
                    if pending:
                        pending.pop(0)()

                def make_pair_tail(l, b0, b1, hpair_t, xt_next):
                    def run():
                        if h_bf:
                            pt = ptp.tile([128, 128], BF, tag="ptp")
                            nc.tensor.transpose(pt[:, :], hpair_t[:, :],
                                                ident_bf[:, :])
                        else:
                            pt = ptp.tile([128, 128], FP32, tag="ptp")
                            nc.tensor.transpose(pt[:, :], hpair_t[:, :],
                                                ident[:, :])
                        ht2 = htp.tile([128, 128], BF, tag="ht2")
                        nc.scalar.copy(ht2[:, :], pt[:, :])
                        vt2 = vt2_sb[l]
                        if l < 2:
                            for half in range(2):
                                oc0 = half * 4
                                poa = pop.tile([128, 512], FP32, tag="po")
                                pob = pop.tile([128, 512], FP32, tag="po")
                                for oc in range(oc0, oc0 + 4):
                                    csl = slice((oc - oc0) * 128,
                                                (oc - oc0 + 1) * 128)
                                    nc.tensor.matmul(
                                        poa[:, csl],
                                        vt2[0:R, oc * 128:(oc + 1) * 128],
                                        ht2[0:R, :],
                                        start=True, stop=True)
                                    nc.tensor.matmul(
                                        pob[:, csl],
                                        vt2[R:128, oc * 128:(oc + 1) * 128],
                                        ht2[R:128, :],
                                        start=True, stop=True)
                                for bx, po_t in ((b0, poa), (b1, pob)):
                                    dst = (xt_next[:, :]
                                           .rearrange("p (k b) -> p k b",
                                                      b=BL)
                                           [:, oc0:oc0 + 4,
                                            bx * 128:bx * 128 + 128])
                                    nc.scalar.activation(
                                        dst,
                                        po_t[:, :].rearrange(
                                            "p (k c) -> p k c", c=128),
                                        mybir.ActivationFunctionType.Relu)
                        else:
                            o_sb0 = osbp.tile([128, D], FP32, tag="osb")
                            o_sb1 = osbp.tile([128, D], FP32, tag="osb")
                            for half in range(2):
                                osl = slice(half * 512, (half + 1) * 512)
                                poa = pop.tile([128, 512], FP32, tag="po")
                                pob = pop.tile([128, 512], FP32, tag="po")
                                nc.tensor.matmul(
                                    poa[:, :], ht2[0:R, :], vt2[0:R, osl],
                                    start=True, stop=True)
                                nc.tensor.matmul(
                                    pob[:, :], ht2[R:128, :],
                                    vt2[R:128, osl],
                                    start=True, stop=True)
                                nc.scalar.copy(o_sb0[:, osl], poa[:, :])
                                nc.vector.tensor_copy(out=o_sb1[:, osl],
                                                      in_=pob[:, :])
                            nc.sync.dma_start(
                                out=out_d[b0 * 128:b0 * 128 + 128, :],
                                in_=o_sb0[:, :])
                            nc.sync.dma_start(
                                out=out_d[b1 * 128:b1 * 128 + 128, :],
                                in_=o_sb1[:, :])
                    return run

                def make_tail(l, bt, h_sb, xt_next, bsl):
                    vt_t = vt_sb[l]
                    items = []

                    if h_dma_t:
                        # keep the transpose off the PE: cast h to bf16 on
                        # DVE, transpose via the DMA xbar. The xbar wants
                        # 128x128 tiles, so pad: only cols 0:64 of h_pad are
                        # written and only rows 0:64 of ht_pad are read.
                        h_pad = hp.tile([128, 128], BF, tag="h_bf")
                        ht_pad = htp.tile([128, 128], BF, tag="ht_pad")
                        ht_sb = ht_pad[0:R, :]

                        def t_transpose():
                            nc.vector.tensor_copy(out=h_pad[:, 0:R], in_=h_sb[:, :])
                            nc.sync.dma_start_transpose(out=ht_pad[:, :],
                                                        in_=h_pad[:, :])
                    else:
                        pt = ptp.tile([R, 128], FP32)
                        ht_tile = htp.tile([R, 128], BF)
                        ht_sb = ht_tile[:, :]

                        def t_transpose():
                            nc.tensor.transpose(pt[:, :], h_sb[:, :], ident[:, :])
                            nc.scalar.copy(ht_sb, pt[:, :])
                    items.append(t_transpose)

                    if l < 2 and v_dma_t:
                        # V GEMM in [b, o] layout with hT stationary (one
                        # LDWEIGHTS), relu to bf16, then DMA-xbar transpose
                        # each [128,128] block into the next layer's lhsT.
                        po = pop.tile([128, D], FP32)
                        for half in range(2):
                            def t_v2a(half=half):
                                osl = slice(half * 512, (half + 1) * 512)
                                nc.tensor.matmul(
                                    po[:, osl], ht_sb[:, :], vt_t[:, osl],
                                    start=True, stop=True)
                            items.append(t_v2a)

                        def t_relu_t():
                            o_bf = osbp.tile([128, D], BF, tag="obf")
                            nc.scalar.activation(
                                o_bf[:, :], po[:, :],
                                mybir.ActivationFunctionType.Relu)
                            for oc in range(NOC):
                                nc.sync.dma_start_transpose(
                                    out=xt_next[:, oc * BL + bt * 128:
                                                oc * BL + bt * 128 + 128],
                                    in_=o_bf[:, oc * 128:(oc + 1) * 128])
                        items.append(t_relu_t)
                    elif l < 2:
                        po = pop.tile([128, NOC * 128], FP32)
                        for oc in range(NOC):
                            def t_v(oc=oc):
                                nc.tensor.matmul(
                                    po[:, oc * 128:(oc + 1) * 128],
                                    vt_t[:, oc * 128:(oc + 1) * 128],
                                    ht_sb[:, :], start=True, stop=True)
                            items.append(t_v)

                        def t_relu():
                            if include_b01:
                                for oc in range(NOC):
                                    nc.scalar.activation(
                                        xt_next[:, oc * BL + bt * 128:
                                                oc * BL + bt * 128 + 128],
                                        po[:, oc * 128:(oc + 1) * 128],
                                        mybir.ActivationFunctionType.Relu,
                                        bias=b01_sb[l][:, oc:oc + 1], scale=1.0)
                            else:
                                nc.scalar.activation(
                                    xt_next[:, :]
                                    .rearrange("p (k b) -> p k b", b=BL)
                                    [:, :, bt * 128:bt * 128 + 128],
                                    po[:, :].rearrange("p (k c) -> p k c", c=128),
                                    mybir.ActivationFunctionType.Relu)
                        items.append(t_relu)
                    else:
                        po = pop.tile([128, D], FP32)
                        for half in range(2):
                            def t_v2(half=half):
                                osl = slice(half * 512, (half + 1) * 512)
                                nc.tensor.matmul(
                                    po[:, osl], ht_sb[:, :], vt_t[:, osl],
                                    start=True, stop=not include_b2)
                                if include_b2:
                                    nc.tensor.matmul(
                                        po[:, osl], ones_bf[:, :], b2_sb[:, osl],
                                        start=False, stop=True)
                            items.append(t_v2)

                        def t_out():
                            o_sb = osbp.tile([128, D], FP32)
                            nc.scalar.copy(o_sb[:, :], po[:, :])
                            nc.sync.dma_start(out=out_d[bsl, :], in_=o_sb[:, :])
                        items.append(t_out)
                    return items

                wm_tiles = None
                xt_l = xt_cur
                for l in range(3):
                    if wm_nodma:
                        wm_tiles = wm_static
                    else:
                        wm_tiles = []
                        dma_eng = nc.gpsimd if wm_gp else nc.sync
                        for k in range(NK):
                            if (l, k) in wm_persist:
                                wm_tiles.append(wm_persist[(l, k)])
                                continue
                            t = wmp.tile([128, R * R], BF, tag="wm")
                            if wm_split > 1:
                                step = (R * R) // wm_split
                                for s0 in range(0, R * R, step):
                                    dma_eng.dma_start(
                                        out=t[:, s0:s0 + step],
                                        in_=wmt[l][k * 128:(k + 1) * 128,
                                                   s0:s0 + step])
                            else:
                                dma_eng.dma_start(
                                    out=t[:, :],
                                    in_=wmt[l][k * 128:(k + 1) * 128, :])
                            wm_tiles.append(t)
                    xt_next = None
                    if l < 2 and ablate != "pe_wm":
                        xt_next = xtp.tile([128, NK * BL], BF, tag="act")

                    hpair = None
                    for bt in range(NBT):
                        bsl = slice(bt * 128, (bt + 1) * 128)
                        xt_b = xt_l

                        def lhs(k, xt_b=xt_b, bt=bt):
                            return xt_b[:, k * BL + bt * 128:
                                        k * BL + bt * 128 + 128]

                        if ko:
                            # k-outer / j-inner: 8 consecutive MMs share the
                            # same stationary lhsT (tests LDW elision)
                            nj = (R * R) // wm_n
                            pws = [pwp.tile([128, wm_n], FP32, tag="pwko",
                                            name=f"pwko{j}")
                                   for j in range(nj)]
                            for k in range(NK):
                                for j in range(nj):
                                    wm_mv = wm_tiles[k][:, j * wm_n:
                                                        (j + 1) * wm_n]
                                    pw_out = pws[j][:, :]
                                    if wm_n > 512:
                                        wm_mv = wm_mv.rearrange(
                                            "p (t n) -> p t n", n=512)
                                        pw_out = pw_out.rearrange(
                                            "p (t n) -> p t n", n=512)
                                    nc.tensor.matmul(
                                        pw_out, lhs(k), wm_mv,
                                        start=(k == 0), stop=(k == NK - 1))
                            if l == 2 and bt == NBT - 1:
                                nc.sync.dma_start(
                                    out=out_d[bsl, :],
                                    in_=xt_b[:, :].bitcast(FP32)[:, 0:D])
                            continue
                        pu = u_sb = h_sb = None
                        if ablate != "pe_wm":
                            pu = pup.tile([128, R], FP32)
                            u_sb = usbp.tile([128, R], FP32)
                            if v_pair:
                                if bt % 2 == 0:
                                    hpair = hp.tile([128, 2 * R],
                                                    BF if h_bf else FP32,
                                                    tag="hpair")
                                # single-step slicing off the tile (NOT a
                                # view-of-a-view) so dependency ranges are
                                # exact
                                h_sb = hpair
                                h_base = (bt % 2) * R
                            else:
                                h_sb = hp.tile([128, R], FP32)
                                h_base = 0
                        if not u_fold:
                            for k in range(NK):
                                nc.tensor.matmul(pu[:, :], lhs(k),
                                                 ut_sb[l][:, k * R:(k + 1) * R],
                                                 start=(k == 0),
                                                 stop=(k == NK - 1))
                            nc.scalar.copy(u_sb[:, :], pu[:, :])
                        nj = (R * R) // wm_n
                        s_per = wm_n // R
                        for j in range(nj):
                            if j == 1 and pending_pair[0] is not None:
                                # deferred pair tail rides here, one chunk
                                # into the next bt's stream, so the DVE lag
                                # on the pair's last chunk never stalls PE
                                pending_pair[0]()
                                pending_pair[0] = None
                            pw = pwp.tile([128, wm_n], FP32)
                            for k in range(NK):
                                wm_mv = wm_tiles[k][:, j * wm_n:(j + 1) * wm_n]
                                pw_out = pw[:, :]
                                if wm_n > 512:
                                    # ISA caps a single AP dim at 512 elements
                                    wm_mv = wm_mv.rearrange(
                                        "p (t n) -> p t n", n=512)
                                    pw_out = pw_out.rearrange(
                                        "p (t n) -> p t n", n=512)
                                nc.tensor.matmul(
                                    pw_out, lhs(k), wm_mv,
                                    start=(k == 0),
                                    stop=(k == NK - 1) and not include_bm)
                                if j == 0 and u_fold and ablate != "pe_wm":
                                    # u GEMM rides the j=0 chunk so its
                                    # LDWEIGHTS hide behind wide matmuls
                                    nc.tensor.matmul(
                                        pu[:, :], lhs(k),
                                        ut_sb[l][:, k * R:(k + 1) * R],
                                        start=(k == 0), stop=(k == NK - 1))
                            if include_bm:
                                nc.tensor.matmul(
                                    pw[:, :], ones_bf[:, :],
                                    bmr_sb[l][:, j * wm_n:(j + 1) * wm_n],
                                    start=False, stop=True)
                            if j == 0 and u_fold and ablate != "pe_wm":
                                nc.scalar.copy(u_sb[:, :], pu[:, :])
                            if ablate == "nodve":
                                nc.vector.tensor_copy(
                                    out=h_sb[:, h_base + j * s_per:
                                             h_base + (j + 1) * s_per],
                                    in_=pw[:, 0:s_per])
                            if ablate != "nodve" and ablate != "pe_wm":
                                # tmp[b, s, r] = w'[b, s, r] * u[b, r]
                                tmp = tmpp.tile([128, wm_n],
                                                BF if tmp_bf else FP32)
                                nc.vector.tensor_tensor(
                                    out=tmp[:, :].rearrange("p (s r) -> p s r", r=R),
                                    in0=pw[:, :].rearrange("p (s r) -> p s r", r=R),
                                    in1=u_sb[:, :].unsqueeze(1)
                                    .broadcast_to([128, s_per, R]),
                                    op=mybir.AluOpType.mult)
                                # h[b, jc*s_per + s] = sum_r tmp[b, s, r]
                                import contextlib as _ctl
                                _lp = (nc.allow_low_precision(
                                    "h is cast to bf16 before the V GEMM "
                                    "anyway; rounding at the reduce is "
                                    "equivalent") if h_bf
                                    else _ctl.nullcontext())
                                with _lp:
                                    nc.vector.tensor_reduce(
                                        out=h_sb[:, h_base + j * s_per:
                                                 h_base + (j + 1) * s_per],
                                        in_=tmp[:, :].rearrange("p (s r) -> p s r", r=R),
                                        axis=mybir.AxisListType.X,
                                        op=mybir.AluOpType.add)
                            for _ in range(wm_n // 512):
                                emit_slot()
                            if j == 0 or j == nj - 1:
                                emit_slot()
                                emit_slot()
                        assert not pending, "tail did not fit in slot budget"
                        if v_pair and ablate is None:
                            # paired-bt tail: one [128,128] transpose covers
                            # two bts; its output lands ht(bt0) on partitions
                            # 0..63 and ht(bt1) on 64..127, so the V GEMMs run
                            # pairwise-concurrent in disjoint PE row groups.
                            if bt % 2 == 0:
                                continue
                            fn = make_pair_tail(l, bt - 1, bt, hpair, xt_next)
                            if pair_defer:
                                pending_pair[0] = fn
                            else:
                                fn()
                            continue
                        if v_batch and ablate is None:
                            # transpose h into the 4-tile group's shared hT
                            # operand; run the V GEMM once per group at
                            # n=512 so LDWEIGHTS amortizes.
                            if bt % 4 == 0:
                                ht4 = ht4p.tile([128 if v_pack else R, 512],
                                                BF, tag="ht4")
                            pt = ptp.tile([R, 128], FP32)
                            nc.tensor.transpose(pt[:, :], h_sb[:, :], ident[:, :])
                            nc.scalar.copy(
                                ht4[0:R, (bt % 4) * 128:(bt % 4 + 1) * 128],
                                pt[:, :])
                            if v_pack:
                                # second copy on partitions 64..127 feeds the
                                # row-group-packed partner matmul
                                nc.scalar.copy(
                                    ht4[R:128, (bt % 4) * 128:(bt % 4 + 1) * 128],
                                    pt[:, :])
                            if bt % 4 == 3:
                                g0 = (bt - 3) * 128
                                if l < 2 and v_pack:
                                    for ocp in range(NOC // 2):
                                        oc0, oc1 = 2 * ocp, 2 * ocp + 1
                                        po0 = pop.tile([128, 512], FP32, tag="po")
                                        po1 = pop.tile([128, 512], FP32, tag="po")
                                        nc.tensor.matmul(
                                            po0[:, :],
                                            vt2_sb[l][0:R, oc0 * 128:(oc0 + 1) * 128],
                                            ht4[0:R, :], start=True, stop=True)
                                        nc.tensor.matmul(
                                            po1[:, :],
                                            vt2_sb[l][R:128, oc1 * 128:(oc1 + 1) * 128],
                                            ht4[R:128, :], start=True, stop=True)
                                        for oc, po in ((oc0, po0), (oc1, po1)):
                                            dst = xt_next[:, oc * BL + g0:
                                                          oc * BL + g0 + 512]
                                            if include_b01:
                                                nc.scalar.activation(
                                                    dst, po[:, :],
                                                    mybir.ActivationFunctionType.Relu,
                                                    bias=b01_sb[l][:, oc:oc + 1],
                                                    scale=1.0)
                                            elif oc % 2 == 1:
                                                # split relus across ACT and DVE
                                                nc.vector.tensor_scalar_max(
                                                    dst, po[:, :], 0.0)
                                            else:
                                                nc.scalar.activation(
                                                    dst, po[:, :],
                                                    mybir.ActivationFunctionType.Relu)
                                elif l < 2:
                                    for oc in range(NOC):
                                        po = pop.tile([128, 512], FP32, tag="po")
                                        nc.tensor.matmul(
                                            po[:, :],
                                            vt_sb[l][:, oc * 128:(oc + 1) * 128],
                                            ht4[:, :], start=True, stop=True)
                                        if include_b01:
                                            nc.scalar.activation(
                                                xt_next[:, oc * BL + g0:
                                                        oc * BL + g0 + 512],
                                                po[:, :],
                                                mybir.ActivationFunctionType.Relu,
                                                bias=b01_sb[l][:, oc:oc + 1],
                                                scale=1.0)
                                        else:
                                            nc.scalar.activation(
                                                xt_next[:, oc * BL + g0:
                                                        oc * BL + g0 + 512],
                                                po[:, :],
                                                mybir.ActivationFunctionType.Relu)
                                elif v_pack:
                                    assert not include_b2
                                    for i4 in range(4):
                                        o_sb = osbp.tile([128, D], FP32)
                                        po0 = pop.tile([128, 512], FP32, tag="po")
                                        po1 = pop.tile([128, 512], FP32, tag="po")
                                        nc.tensor.matmul(
                                            po0[:, :],
                                            ht4[0:R, i4 * 128:(i4 + 1) * 128],
                                            vt2_sb[l][0:R, 0:512],
                                            start=True, stop=True)
                                        nc.tensor.matmul(
                                            po1[:, :],
                                            ht4[R:128, i4 * 128:(i4 + 1) * 128],
                                            vt2_sb[l][R:128, 512:1024],
                                            start=True, stop=True)
                                        nc.scalar.copy(o_sb[:, 0:512], po0[:, :])
                                        nc.vector.tensor_copy(
                                            out=o_sb[:, 512:1024], in_=po1[:, :])
                                        row0 = (bt - 3 + i4) * 128
                                        nc.sync.dma_start(
                                            out=out_d[row0:row0 + 128, :],
                                            in_=o_sb[:, :])
                                else:
                                    for i4 in range(4):
                                        o_sb = osbp.tile([128, D], FP32)
                                        for half in range(2):
                                            osl = slice(half * 512,
                                                        (half + 1) * 512)
                                            po = pop.tile([128, 512], FP32,
                                                          tag="po")
                                            nc.tensor.matmul(
                                                po[:, :],
                                                ht4[:, i4 * 128:(i4 + 1) * 128],
                                                vt_sb[l][:, osl],
                                                start=True, stop=not include_b2)
                                            if include_b2:
                                                nc.tensor.matmul(
                                                    po[:, :], ones_bf[:, :],
                                                    b2_sb[:, osl],
                                                    start=False, stop=True)
                                            nc.scalar.copy(o_sb[:, osl],
                                                           po[:, :])
                                        row0 = (bt - 3 + i4) * 128
                                        nc.sync.dma_start(
                                            out=out_d[row0:row0 + 128, :],
                                            in_=o_sb[:, :])
                            continue
                        if ablate == "pe_wm":
                            if l == 2 and bt == NBT - 1:
                                # keep the output write so the graph has one
                                nc.sync.dma_start(
                                    out=out_d[bsl, :],
                                    in_=xt_b[:, :].bitcast(FP32)[:, 0:D])
                            continue
                        pending = make_tail(l, bt, h_sb, xt_next, bsl)
                        if not pipeline:
                            for fn in pending:
                                fn()
                            pending = []
                    if xt_next is not None:
                        xt_l = xt_next

                if pending_pair[0] is not None:
                    pending_pair[0]()
                    pending_pair[0] = None
                for fn in pending:
                    fn()
                pending = []
    nc.compile()
    if dedup_ldw:
        n = dedup_ldweights(nc)
        print(f"dedup_ldweights: removed {n}")
    if batch_updates:
        n = batch_mm_updates(nc, every=batch_updates)
        print(f"batch_mm_updates: batched {n}")
    return nc


# ---------------------------------------------------------------------------
# host side
# ---------------------------------------------------------------------------

def _prep_static(Wm, bm, U, V, b):
    """Host-side layout prep of one layer's replicated params."""
    Wm = np.asarray(Wm, dtype=np.float32)
    # rows j' = s*64 + r  <->  original j = r*64 + s ; then transpose -> [k, j']
    wmt = np.ascontiguousarray(
        Wm.reshape(R, R, D).transpose(1, 0, 2).reshape(R * R, D).T).astype(BF16)
    utm = np.ascontiguousarray(np.asarray(U, dtype=np.float32).T).astype(BF16)
    vtm = np.ascontiguousarray(np.asarray(V, dtype=np.float32).T).astype(BF16)
    return wmt, utm, vtm


_CACHE = {}


def _get_compiled(flags):
    if flags not in _CACHE:
        _CACHE[flags] = build_apg(*flags)
    return _CACHE[flags]


def _make_in_maps(x, layers, flags, reps):
    include_bm, include_b01, include_b2 = flags
    x = np.asarray(x, dtype=np.float32)
    shared = {}
    for l, (Wm, bm, U, V, b) in enumerate(layers):
        wmt, utm, vtm = _prep_static(Wm, bm, U, V, b)
        shared[f"wmt{l}"] = wmt
        shared[f"ut{l}"] = utm
        shared[f"vt{l}"] = vtm
        if include_bm:
            shared[f"bmr{l}"] = np.asarray(bm, np.float32).reshape(R, R).T \
                .reshape(1, R * R).astype(BF16)
        if include_b01 and l < 2:
            shared[f"b{l}c"] = np.ascontiguousarray(
                np.asarray(b, np.float32).reshape(NOC, 128).T)
        if include_b2 and l == 2:
            shared["b2r"] = np.asarray(b, np.float32).reshape(1, D).astype(BF16)
    shared["reps"] = np.array([[reps]], dtype=np.uint32)
    in_maps = []
    for i in range(NCORES):
        m = dict(shared)
        xs = x[i * BL:(i + 1) * BL, :]
        m["xt"] = np.ascontiguousarray(xs.T).astype(BF16)
        in_maps.append(m)
    return in_maps


_RUNNER_CACHE = {}


def _get_runner(flags):
    """Jit-once PJRT runner for the compiled module (same execution path as
    bass_utils.run_bass_kernel_spmd's axon redirect through bass2jax, but
    cached so repeat kernel() calls skip re-trace/re-compile)."""
    if flags in _RUNNER_CACHE:
        return _RUNNER_CACHE[flags]
    import jax
    from jax.sharding import Mesh, PartitionSpec, NamedSharding
    from jax.experimental.shard_map import shard_map
    from concourse import bass2jax

    nc = _get_compiled(flags)
    bass2jax.install_neuronx_cc_hook()
    partition_name = nc.partition_id_tensor.name if nc.partition_id_tensor else None
    in_names, out_names, out_avals, zero_outs = [], [], [], []
    for alloc in nc.m.functions[0].allocations:
        if not isinstance(alloc, mybir.MemoryLocationSet):
            continue
        name = alloc.memorylocations[0].name
        if alloc.kind == "ExternalInput":
            if name != partition_name:
                in_names.append(name)
        elif alloc.kind == "ExternalOutput":
            out_names.append(name)
            shape = tuple(alloc.tensor_shape)
            dtype = mybir.dt.np(alloc.dtype)
            out_avals.append(jax.core.ShapedArray(shape, dtype))
            zero_outs.append(np.zeros(shape, dtype))
    n_params = len(in_names)
    all_in_names = list(in_names) + list(out_names)
    if partition_name is not None:
        all_in_names.append(partition_name)

    def _body(*args):
        operands = list(args)
        if partition_name is not None:
            operands = operands + [bass2jax.partition_id_tensor()]
        outs = bass2jax._bass_exec_p.bind(
            *operands, out_avals=tuple(out_avals), in_names=tuple(all_in_names),
            out_names=tuple(out_names), lowering_input_output_aliases=(),
            sim_require_finite=True, sim_require_nnan=True, nc=nc)
        return tuple(outs)

    devices = jax.devices()[:NCORES]
    mesh = Mesh(np.asarray(devices), ("core",))
    in_specs = (PartitionSpec("core"),) * (n_params + len(out_names))
    out_specs = (PartitionSpec("core"),) * len(out_names)
    fn = jax.jit(shard_map(_body, mesh=mesh, in_specs=in_specs,
                           out_specs=out_specs, check_rep=False))
    sh = NamedSharding(mesh, PartitionSpec("core"))

    # Outputs are constant zero-filled donor buffers — upload once.
    zero_dev = [jax.device_put(np.concatenate([z] * NCORES, axis=0), sh)
                for z in zero_outs]
    dev_cache = {}

    def _put_cached(name, arr):
        import hashlib
        key = (name, arr.shape, arr.dtype.str,
               hashlib.blake2b(np.ascontiguousarray(arr).tobytes(),
                               digest_size=16).hexdigest())
        if key not in dev_cache:
            if len(dev_cache) > 64:
                dev_cache.clear()
            dev_cache[key] = jax.device_put(arr, sh)
        return dev_cache[key]

    def run(in_maps):
        dev = [_put_cached(name,
                           np.concatenate([np.asarray(m[name]) for m in in_maps],
                                          axis=0))
               for name in in_names]
        outs = fn(*(dev + zero_dev))
        jax.block_until_ready(outs)
        return {name: np.asarray(outs[i]) for i, name in enumerate(out_names)}

    _RUNNER_CACHE[flags] = run
    return run


def kernel(x, Wm0, bm0, U0, V0, b0, Wm1, bm1, U1, V1, b1,
           Wm2, bm2, U2, V2, b2):
    layers = [(Wm0, bm0, U0, V0, b0), (Wm1, bm1, U1, V1, b1),
              (Wm2, bm2, U2, V2, b2)]
    flags = (
        any(np.any(np.asarray(t[1], np.float32)) for t in layers),
        any(np.any(np.asarray(t[4], np.float32)) for t in layers[:2]),
        bool(np.any(np.asarray(layers[2][4], np.float32))),
    )
    run = _get_runner(flags)
    in_maps = _make_in_maps(x, layers, flags, reps=1)
    res = None
    for attempt in range(3):
        try:
            res = run(in_maps)
            # transient device flakes can also corrupt silently (NaNs in
            # the output); the math here is all-finite by construction
            if np.isfinite(res["out"]).all():
                break
        except Exception:
            if attempt == 2:
                raise
        # retry on fresh runner + device buffers
        _RUNNER_CACHE.pop(flags, None)
        run = _get_runner(flags)
    # res["out"] is the concatenation of the 8 per-core [BL, D] shards
    return np.ascontiguousarray(res["out"]).astype(np.float32)



# revision 47
# speedup vs baseline: 1.0198x; 1.0058x over previous
"""Trainium2 Bass kernel for nn_APG_MLP_Layer (3-layer APG hyper-network MLP).

Reference computation per layer (B=8192, din=dout=1024, RANK=64):
    w = (x @ Wm.T + bm).reshape(B, 64, 64)   # per-sample generated weights
    u = x @ U.T                              # [B, 64]
    h = einsum('br,brs->bs', u, w)           # per-sample vec-mat product
    out = relu?(h @ V.T + b)

Sharding: data-parallel over batch across 8 NeuronCores (1024 rows/core);
static params replicated.

Device mapping (per core, per 128-row batch tile):
  - Wm GEMM dominates (8192x1024x4096 per layer). Host pre-transposes all
    static operands and reorders Wm rows to j' = s*64 + r so that each PSUM
    chunk [128b, 512] holds w'[b, s_block(8), r(64)] with r contiguous.
  - The einsum contraction is then one DVE tensor_tensor multiply with u
    broadcast over s (step-0 AP) + one inner-axis tensor_reduce -> h[b, s].
  - h (bf16) is PE-transposed in PAIRS of batch tiles: one [128,128]
    transpose covers two bts and lands ht(bt0) on partitions 0..63 /
    ht(bt1) on 64..127, so the V GEMMs run pairwise-concurrent in disjoint
    PE row groups (vt2 = V.T duplicated on both partition halves). Layers
    0/1 compute outT[o, b] (ReLU'd output directly the next layer's lhsT);
    layer 2 computes out[b, o]. The pair tail is emitted one j-chunk into
    the next bt's wm stream so the DVE lag never stalls the PE.
  - All matmuls run in bf16 (fp32 accumulate in PSUM).

The kernel has a runtime `reps` loop (register-bound For_i) so the same NEFF
serves correctness (reps=1) and steady-state timing (reps=R, slope method).
"""

import numpy as np
import ml_dtypes

import concourse.bass as bass
import concourse.mybir as mybir
from concourse import bacc
from concourse.tile import TileContext
from concourse.masks import make_identity

BF16 = ml_dtypes.bfloat16
FP32 = mybir.dt.float32
BF = mybir.dt.bfloat16

B = 8192
NCORES = 8
BL = B // NCORES          # 1024 rows per core
D = 1024                  # all layer dims
R = 64                    # rank
NBT = BL // 128           # batch tiles per core (8)
NK = D // 128             # k chunks (8)
NJ = (R * R) // 512       # j chunks of 512 (8)
NOC = D // 128            # output chunks (8)


def _ldw_key(inst):
    a = inst.ins[0]
    if getattr(a, "dynamic_ap_info", None) is not None:
        return None
    return (a.memref, a.offset, str(a.ap), str(a.dtype),
            inst.tile_position, inst.tile_size, str(inst.perf_mode),
            inst.is_transpose)


def dedup_ldweights(nc):
    """Post-compile pass: drop InstLdweights that reload the exact weights
    already resident in the PE array (same AP, no intervening clobber).
    The lowered InstMatmults are non-self-loading (ldweights=False), so a
    dropped redundant load is semantics-preserving. LDWs carrying semaphore
    waits are kept."""
    removed = 0
    for f in nc.m.functions:
        for bb in f.blocks:
            insts = list(bb.instructions)
            cur = None
            out = []
            changed = False
            for inst in insts:
                tn = type(inst).__name__
                if str(inst.engine) != "EngineType.PE":
                    out.append(inst)
                    continue
                if tn == "InstLdweights":
                    key = _ldw_key(inst)
                    if key is not None and key == cur and not inst.has_wait():
                        removed += 1
                        changed = True
                        continue
                    cur = key
                    out.append(inst)
                elif tn in ("InstMatmult", "InstMatmultMx"):
                    if getattr(inst, "is_transpose", None):
                        cur = None
                    out.append(inst)
                else:
                    cur = None
                    out.append(inst)
            if changed:
                bb.instructions = out
    return removed


def batch_mm_updates(nc, every=8):
    """Timing experiment: batch per-MM semaphore increments — keep one inc of
    value `every` on each every-th MM, drop the rest. Only valid when nothing
    waits on intermediate values of the PE semaphore (pe_wm ablations)."""
    import concourse.mybir as mybir
    nbat = 0
    for f in nc.m.functions:
        for bb in f.blocks:
            insts = list(bb.instructions)
            mms = [i for i in insts
                   if type(i).__name__ == "InstMatmult"
                   and str(i.engine) == "EngineType.PE"
                   and i.sync_info is not None
                   and len(i.sync_info.on_update) == 1
                   and not i.sync_info.on_wait]
            if len(mms) < every:
                continue
            # group by target semaphore id
            from collections import defaultdict
            by_sem = defaultdict(list)
            for i in mms:
                u = i.sync_info.on_update[0]
                if u.update_mode == "sem-inc" and u.update_value == 1:
                    by_sem[u.id].append(i)
            for sem, lst in by_sem.items():
                n = len(lst)
                nfull = n // every
                for idx, inst in enumerate(lst):
                    gi = idx // every
                    if gi >= nfull:
                        continue  # leave the remainder with inc 1
                    si = inst.sync_info
                    if (idx + 1) % every == 0:
                        u = si.on_update[0]
                        u.update_value = every
                        inst.sync_info = si
                        nbat += 1
                    else:
                        si.on_update = []
                        inst.sync_info = si
    return nbat


def build_apg(include_bm=False, include_b01=False, include_b2=False,
              reps_loop=True, pipeline=False, u_fold=True, v_dma_t=False,
              h_dma_t=False, wm_gp=False, v_batch=False, ablate=None,
              loop_kwargs=None, tmp_bufs=4, wm_bufs=16, act_bufs=3,
              wm_n=512, pw_bufs=None, persist=(0, 0, 0), persist_xt=False,
              osb_bufs=2, v_pack=False, po_bufs=None, dedup_ldw=False,
              batch_updates=0, xt_split=False, wm_split=1, v_pair=True,
              h_bf=True, pair_defer=True, tmp_bf=False):
    """Build + compile the Bass module. Returns (nc, names) where names lists
    the DRAM input tensor names in declaration order."""
    import contextlib
    wm_nodma = ablate == "pe_wm_nodma"
    if wm_nodma:
        ablate = "pe_wm"
    ko = ablate == "pe_wm_ko"
    if ko:
        ablate = "pe_wm"
    if v_pack:
        v_batch = True
    if include_b01 or include_b2:
        # the paired tail doesn't apply the output biases; use the general
        # per-bt tail (which does) when they are present
        v_pair = False
    nc = bacc.Bacc("TRN2", target_bir_lowering=False, debug=False,
                   num_devices=NCORES)

    xt = nc.dram_tensor("xt", [D, BL], BF, kind="ExternalInput")
    wmt = [nc.dram_tensor(f"wmt{l}", [D, R * R], BF, kind="ExternalInput")
           for l in range(3)]
    ut = [nc.dram_tensor(f"ut{l}", [D, R], BF, kind="ExternalInput")
          for l in range(3)]
    vt = [nc.dram_tensor(f"vt{l}", [R, D], BF, kind="ExternalInput")
          for l in range(3)]
    bm_row = b01_col = b2_row = None
    if include_bm:
        bm_row = [nc.dram_tensor(f"bmr{l}", [1, R * R], BF, kind="ExternalInput")
                  for l in range(3)]
    if include_b01:
        b01_col = [nc.dram_tensor(f"b{l}c", [128, NOC], FP32, kind="ExternalInput")
                   for l in range(2)]
    if include_b2:
        b2_row = nc.dram_tensor("b2r", [1, D], BF, kind="ExternalInput")
    reps_t = None
    if reps_loop:
        reps_t = nc.dram_tensor("reps", [1, 1], mybir.dt.uint32,
                                kind="ExternalInput")
    out_d = nc.dram_tensor("out", [BL, D], FP32, kind="ExternalOutput")

    with TileContext(nc) as tc:
        with (
            tc.tile_pool(name="const", bufs=1) as constp,
            tc.tile_pool(name="xt", bufs=2) as xtp,
            tc.tile_pool(name="wm", bufs=wm_bufs) as wmp,
            tc.tile_pool(name="usb", bufs=act_bufs) as usbp,
            tc.tile_pool(name="h", bufs=act_bufs) as hp,
            tc.tile_pool(name="ht", bufs=act_bufs) as htp,
            tc.tile_pool(name="tmp", bufs=tmp_bufs) as tmpp,
            tc.tile_pool(name="osb", bufs=osb_bufs) as osbp,
            tc.tile_pool(name="pw", bufs=(pw_bufs if pw_bufs is not None
                                          else 2 if v_pair
                                          else (3 if v_pack else 4)
                                          if wm_n == 512 else 2),
                         space="PSUM") as pwp,
            tc.tile_pool(name="pu", bufs=1, space="PSUM") as pup,
            tc.tile_pool(name="pt", bufs=1, space="PSUM") as ptp,
            tc.tile_pool(name="po", bufs=(po_bufs if po_bufs is not None
                                          else 4 if v_pair
                                          else 3 if v_pack
                                          else 2 if v_batch else 1),
                         space="PSUM") as pop,
            tc.tile_pool(name="ht4", bufs=2) as ht4p,
        ):
            # ---- constants (loaded once, outside the reps loop) ----
            ident = constp.tile([128, 128], FP32, tag="ident")
            make_identity(nc, ident[:, :])
            ident_bf = None
            if h_bf:
                ident_bf = constp.tile([128, 128], BF, tag="identbf")
                make_identity(nc, ident_bf[:, :])
            vt_sb = []
            vt2_sb = []
            for l in range(3):
                if v_pack or v_pair:
                    # V.T duplicated on both partition halves so K=64 V-GEMMs
                    # can run pairwise in disjoint PE row groups
                    t2 = constp.tile([128, D], BF, tag=f"vt2_{l}")
                    nc.sync.dma_start(out=t2[0:R, :], in_=vt[l][:, :])
                    nc.sync.dma_start(out=t2[R:128, :], in_=vt[l][:, :])
                    vt2_sb.append(t2)
                    vt_sb.append(t2)
                else:
                    t = constp.tile([R, D], BF, tag=f"vt{l}")
                    nc.sync.dma_start(out=t[:, :], in_=vt[l][:, :])
                    vt_sb.append(t)
                    vt2_sb.append(None)
            ut_sb = []
            for l in range(3):
                # [128, NK*R]: column block k holds U_l.T rows k*128..k*128+127
                t = constp.tile([128, NK * R], BF, tag=f"ut{l}")
                nc.sync.dma_start(
                    out=t[:, :].rearrange("p (k r) -> p k r", r=R),
                    in_=ut[l][:, :].rearrange("(k p) r -> p k r", p=128))
                ut_sb.append(t)
            ones_bf = None
            if include_bm or include_b2:
                ones_bf = constp.tile([1, 128], BF, tag="ones")
                nc.vector.memset(ones_bf[:, :], 1.0)
            bmr_sb = []
            if include_bm:
                for l in range(3):
                    t = constp.tile([1, R * R], BF, tag=f"bmr{l}")
                    nc.sync.dma_start(out=t[:, :], in_=bm_row[l][:, :])
                    bmr_sb.append(t)
            b01_sb = []
            if include_b01:
                for l in range(2):
                    t = constp.tile([128, NOC], FP32, tag=f"b01_{l}")
                    nc.sync.dma_start(out=t[:, :], in_=b01_col[l][:, :])
                    b01_sb.append(t)
            b2_sb = None
            if include_b2:
                b2_sb = constp.tile([1, D], BF, tag="b2")
                nc.sync.dma_start(out=b2_sb[:, :], in_=b2_row[:, :])

            # runtime rep count on all engines
            if reps_loop:
                regs = nc.alloc_registers("reps_regs", mybir.ALL_ENGINES)
                nc.regs_load(regs, reps_t[0:1, 0:1])
                reps_val = nc.snap(regs, donate=True, min_val=1, max_val=1 << 20)
                loop_cm = tc.For_i(0, reps_val, 1, **(loop_kwargs or {}))
            else:
                loop_cm = contextlib.nullcontext()

            wm_static = None
            if wm_nodma:
                # one wm tile set loaded outside the reps loop, reused for
                # all layers (timing ablation only — results are wrong)
                wm_static = []
                for k in range(NK):
                    t = constp.tile([128, R * R], BF, tag=f"wmstat{k}")
                    nc.sync.dma_start(out=t[:, :],
                                      in_=wmt[0][k * 128:(k + 1) * 128, :])
                    wm_static.append(t)

            # weight-stationary: persist the first persist[l] wm tiles of each
            # layer in SBUF (loaded once, outside the reps loop)
            wm_persist = {}
            for l in range(3):
                for k in range(persist[l]):
                    t = constp.tile([128, R * R], BF, tag=f"wmp{l}_{k}")
                    nc.sync.dma_start(out=t[:, :],
                                      in_=wmt[l][k * 128:(k + 1) * 128, :])
                    wm_persist[(l, k)] = t
            xt_static = None
            if persist_xt:
                xt_static = constp.tile([128, NK * BL], BF, tag="xt0")
                nc.sync.dma_start(
                    out=xt_static[:, :].rearrange("p (k b) -> p k b", b=BL),
                    in_=xt[:, :].rearrange("(k p) b -> p k b", p=128))

            with loop_cm:
                # activations (lhsT layout): [128, NK*BL] bf16; col block k
                # holds x.T rows k*128..k*128+127 (i.e. x cols), b along free.
                if persist_xt:
                    xt_cur = xt_static
                else:
                    xt_cur = xtp.tile([128, NK * BL], BF, tag="act")
                    if xt_split:
                        # per-k-chunk DMAs so bt0's first matmuls only wait
                        # on the first 256KB, not the whole 2MB
                        for k in range(NK):
                            nc.sync.dma_start(
                                out=xt_cur[:, k * BL:(k + 1) * BL],
                                in_=xt[k * 128:(k + 1) * 128, :])
                    else:
                        nc.sync.dma_start(
                            out=xt_cur[:, :].rearrange("p (k b) -> p k b", b=BL),
                            in_=xt[:, :].rearrange("(k p) b -> p k b", p=128))

                # Software pipeline over (layer, batch-tile): each
                # iteration's tail (h transpose + V GEMM + relu/output) is
                # emitted interleaved into the NEXT iteration's wm-GEMM
                # stream so its small LDWEIGHTS-bound matmuls hide behind
                # the 512-column wm matmuls. `pending` holds the tail
                # closures of the previous (l, bt).
                pending = []
                pending_pair = []

                def emit_slot():
                    if pending:
                        pending.pop(0)()

                def make_pair_tail(l, b0, b1, hpair_t, xt_next):
                    state = {}

                    def run_a():
                        # slot A: transpose + PSUM->SBUF copy only, so the
                        # V matmuls emitted a chunk later never head-of-line
                        # block the PE queue waiting on the ACT copy
                        if h_bf:
                            pt = ptp.tile([128, 128], BF, tag="ptp")
                            nc.tensor.transpose(pt[:, :], hpair_t[:, :],
                                                ident_bf[:, :])
                        else:
                            pt = ptp.tile([128, 128], FP32, tag="ptp")
                            nc.tensor.transpose(pt[:, :], hpair_t[:, :],
                                                ident[:, :])
                        ht2 = htp.tile([128, 128], BF, tag="ht2")
                        nc.scalar.copy(ht2[:, :], pt[:, :])
                        state["ht2"] = ht2

                    def run_b():
                        ht2 = state["ht2"]
                        vt2 = vt2_sb[l]
                        if l < 2:
                            for half in range(2):
                                oc0 = half * 4
                                poa = pop.tile([128, 512], FP32, tag="po")
                                pob = pop.tile([128, 512], FP32, tag="po")
                                for oc in range(oc0, oc0 + 4):
                                    csl = slice((oc - oc0) * 128,
                                                (oc - oc0 + 1) * 128)
                                    nc.tensor.matmul(
                                        poa[:, csl],
                                        vt2[0:R, oc * 128:(oc + 1) * 128],
                                        ht2[0:R, :],
                                        start=True, stop=True)
                                    nc.tensor.matmul(
                                        pob[:, csl],
                                        vt2[R:128, oc * 128:(oc + 1) * 128],
                                        ht2[R:128, :],
                                        start=True, stop=True)
                                for bx, po_t in ((b0, poa), (b1, pob)):
                                    dst = (xt_next[:, :]
                                           .rearrange("p (k b) -> p k b",
                                                      b=BL)
                                           [:, oc0:oc0 + 4,
                                            bx * 128:bx * 128 + 128])
                                    nc.scalar.activation(
                                        dst,
                                        po_t[:, :].rearrange(
                                            "p (k c) -> p k c", c=128),
                                        mybir.ActivationFunctionType.Relu)
                        else:
                            o_sb0 = osbp.tile([128, D], FP32, tag="osb")
                            o_sb1 = osbp.tile([128, D], FP32, tag="osb")
                            for half in range(2):
                                osl = slice(half * 512, (half + 1) * 512)
                                poa = pop.tile([128, 512], FP32, tag="po")
                                pob = pop.tile([128, 512], FP32, tag="po")
                                nc.tensor.matmul(
                                    poa[:, :], ht2[0:R, :], vt2[0:R, osl],
                                    start=True, stop=True)
                                nc.tensor.matmul(
                                    pob[:, :], ht2[R:128, :],
                                    vt2[R:128, osl],
                                    start=True, stop=True)
                                nc.scalar.copy(o_sb0[:, osl], poa[:, :])
                                nc.vector.tensor_copy(out=o_sb1[:, osl],
                                                      in_=pob[:, :])
                            nc.sync.dma_start(
                                out=out_d[b0 * 128:b0 * 128 + 128, :],
                                in_=o_sb0[:, :])
                            nc.sync.dma_start(
                                out=out_d[b1 * 128:b1 * 128 + 128, :],
                                in_=o_sb1[:, :])
                    return [run_a, run_b]

                def make_tail(l, bt, h_sb, xt_next, bsl):
                    vt_t = vt_sb[l]
                    items = []

                    if h_dma_t:
                        # keep the transpose off the PE: cast h to bf16 on
                        # DVE, transpose via the DMA xbar. The xbar wants
                        # 128x128 tiles, so pad: only cols 0:64 of h_pad are
                        # written and only rows 0:64 of ht_pad are read.
                        h_pad = hp.tile([128, 128], BF, tag="h_bf")
                        ht_pad = htp.tile([128, 128], BF, tag="ht_pad")
                        ht_sb = ht_pad[0:R, :]

                        def t_transpose():
                            nc.vector.tensor_copy(out=h_pad[:, 0:R], in_=h_sb[:, :])
                            nc.sync.dma_start_transpose(out=ht_pad[:, :],
                                                        in_=h_pad[:, :])
                    else:
                        pt = ptp.tile([R, 128], FP32)
                        ht_tile = htp.tile([R, 128], BF)
                        ht_sb = ht_tile[:, :]

                        def t_transpose():
                            nc.tensor.transpose(pt[:, :], h_sb[:, :], ident[:, :])
                            nc.scalar.copy(ht_sb, pt[:, :])
                    items.append(t_transpose)

                    if l < 2 and v_dma_t:
                        # V GEMM in [b, o] layout with hT stationary (one
                        # LDWEIGHTS), relu to bf16, then DMA-xbar transpose
                        # each [128,128] block into the next layer's lhsT.
                        po = pop.tile([128, D], FP32)
                        for half in range(2):
                            def t_v2a(half=half):
                                osl = slice(half * 512, (half + 1) * 512)
                                nc.tensor.matmul(
                                    po[:, osl], ht_sb[:, :], vt_t[:, osl],
                                    start=True, stop=True)
                            items.append(t_v2a)

                        def t_relu_t():
                            o_bf = osbp.tile([128, D], BF, tag="obf")
                            nc.scalar.activation(
                                o_bf[:, :], po[:, :],
                                mybir.ActivationFunctionType.Relu)
                            for oc in range(NOC):
                                nc.sync.dma_start_transpose(
                                    out=xt_next[:, oc * BL + bt * 128:
                                                oc * BL + bt * 128 + 128],
                                    in_=o_bf[:, oc * 128:(oc + 1) * 128])
                        items.append(t_relu_t)
                    elif l < 2:
                        po = pop.tile([128, NOC * 128], FP32)
                        for oc in range(NOC):
                            def t_v(oc=oc):
                                nc.tensor.matmul(
                                    po[:, oc * 128:(oc + 1) * 128],
                                    vt_t[:, oc * 128:(oc + 1) * 128],
                                    ht_sb[:, :], start=True, stop=True)
                            items.append(t_v)

                        def t_relu():
                            if include_b01:
                                for oc in range(NOC):
                                    nc.scalar.activation(
                                        xt_next[:, oc * BL + bt * 128:
                                                oc * BL + bt * 128 + 128],
                                        po[:, oc * 128:(oc + 1) * 128],
                                        mybir.ActivationFunctionType.Relu,
                                        bias=b01_sb[l][:, oc:oc + 1], scale=1.0)
                            else:
                                nc.scalar.activation(
                                    xt_next[:, :]
                                    .rearrange("p (k b) -> p k b", b=BL)
                                    [:, :, bt * 128:bt * 128 + 128],
                                    po[:, :].rearrange("p (k c) -> p k c", c=128),
                                    mybir.ActivationFunctionType.Relu)
                        items.append(t_relu)
                    else:
                        po = pop.tile([128, D], FP32)
                        for half in range(2):
                            def t_v2(half=half):
                                osl = slice(half * 512, (half + 1) * 512)
                                nc.tensor.matmul(
                                    po[:, osl], ht_sb[:, :], vt_t[:, osl],
                                    start=True, stop=not include_b2)
                                if include_b2:
                                    nc.tensor.matmul(
                                        po[:, osl], ones_bf[:, :], b2_sb[:, osl],
                                        start=False, stop=True)
                            items.append(t_v2)

                        def t_out():
                            o_sb = osbp.tile([128, D], FP32)
                            nc.scalar.copy(o_sb[:, :], po[:, :])
                            nc.sync.dma_start(out=out_d[bsl, :], in_=o_sb[:, :])
                        items.append(t_out)
                    return items

                wm_tiles = None
                xt_l = xt_cur
                for l in range(3):
                    if wm_nodma:
                        wm_tiles = wm_static
                    else:
                        wm_tiles = []
                        dma_eng = nc.gpsimd if wm_gp else nc.sync
                        for k in range(NK):
                            if (l, k) in wm_persist:
                                wm_tiles.append(wm_persist[(l, k)])
                                continue
                            t = wmp.tile([128, R * R], BF, tag="wm")
                            if wm_split > 1:
                                step = (R * R) // wm_split
                                for s0 in range(0, R * R, step):
                                    dma_eng.dma_start(
                                        out=t[:, s0:s0 + step],
                                        in_=wmt[l][k * 128:(k + 1) * 128,
                                                   s0:s0 + step])
                            else:
                                dma_eng.dma_start(
                                    out=t[:, :],
                                    in_=wmt[l][k * 128:(k + 1) * 128, :])
                            wm_tiles.append(t)
                    xt_next = None
                    if l < 2 and ablate != "pe_wm":
                        xt_next = xtp.tile([128, NK * BL], BF, tag="act")

                    hpair = None
                    for bt in range(NBT):
                        bsl = slice(bt * 128, (bt + 1) * 128)
                        xt_b = xt_l

                        def lhs(k, xt_b=xt_b, bt=bt):
                            return xt_b[:, k * BL + bt * 128:
                                        k * BL + bt * 128 + 128]

                        if ko:
                            # k-outer / j-inner: 8 consecutive MMs share the
                            # same stationary lhsT (tests LDW elision)
                            nj = (R * R) // wm_n
                            pws = [pwp.tile([128, wm_n], FP32, tag="pwko",
                                            name=f"pwko{j}")
                                   for j in range(nj)]
                            for k in range(NK):
                                for j in range(nj):
                                    wm_mv = wm_tiles[k][:, j * wm_n:
                                                        (j + 1) * wm_n]
                                    pw_out = pws[j][:, :]
                                    if wm_n > 512:
                                        wm_mv = wm_mv.rearrange(
                                            "p (t n) -> p t n", n=512)
                                        pw_out = pw_out.rearrange(
                                            "p (t n) -> p t n", n=512)
                                    nc.tensor.matmul(
                                        pw_out, lhs(k), wm_mv,
                                        start=(k == 0), stop=(k == NK - 1))
                            if l == 2 and bt == NBT - 1:
                                nc.sync.dma_start(
                                    out=out_d[bsl, :],
                                    in_=xt_b[:, :].bitcast(FP32)[:, 0:D])
                            continue
                        pu = u_sb = h_sb = None
                        if ablate != "pe_wm":
                            pu = pup.tile([128, R], FP32)
                            u_sb = usbp.tile([128, R], FP32)
                            if v_pair:
                                if bt % 2 == 0:
                                    hpair = hp.tile([128, 2 * R],
                                                    BF if h_bf else FP32,
                                                    tag="hpair")
                                # single-step slicing off the tile (NOT a
                                # view-of-a-view) so dependency ranges are
                                # exact
                                h_sb = hpair
                                h_base = (bt % 2) * R
                            else:
                                h_sb = hp.tile([128, R], FP32)
                                h_base = 0
                        if not u_fold:
                            for k in range(NK):
                                nc.tensor.matmul(pu[:, :], lhs(k),
                                                 ut_sb[l][:, k * R:(k + 1) * R],
                                                 start=(k == 0),
                                                 stop=(k == NK - 1))
                            nc.scalar.copy(u_sb[:, :], pu[:, :])
                        nj = (R * R) // wm_n
                        s_per = wm_n // R
                        for j in range(nj):
                            if j in (1, 2) and pending_pair:
                                # deferred pair-tail slots ride here, one and
                                # two chunks into the next bt's stream, so
                                # neither the DVE lag on the pair's last
                                # chunk nor the ACT ht copy ever stalls PE
                                pending_pair.pop(0)()
                            pw = pwp.tile([128, wm_n], FP32)
                            for k in range(NK):
                                wm_mv = wm_tiles[k][:, j * wm_n:(j + 1) * wm_n]
                                pw_out = pw[:, :]
                                if wm_n > 512:
                                    # ISA caps a single AP dim at 512 elements
                                    wm_mv = wm_mv.rearrange(
                                        "p (t n) -> p t n", n=512)
                                    pw_out = pw_out.rearrange(
                                        "p (t n) -> p t n", n=512)
                                nc.tensor.matmul(
                                    pw_out, lhs(k), wm_mv,
                                    start=(k == 0),
                                    stop=(k == NK - 1) and not include_bm)
                                if j == 0 and u_fold and ablate != "pe_wm":
                                    # u GEMM rides the j=0 chunk so its
                                    # LDWEIGHTS hide behind wide matmuls
                                    nc.tensor.matmul(
                                        pu[:, :], lhs(k),
                                        ut_sb[l][:, k * R:(k + 1) * R],
                                        start=(k == 0), stop=(k == NK - 1))
                            if include_bm:
                                nc.tensor.matmul(
                                    pw[:, :], ones_bf[:, :],
                                    bmr_sb[l][:, j * wm_n:(j + 1) * wm_n],
                                    start=False, stop=True)
                            if j == 0 and u_fold and ablate != "pe_wm":
                                nc.scalar.copy(u_sb[:, :], pu[:, :])
                            if ablate == "nodve":
                                nc.vector.tensor_copy(
                                    out=h_sb[:, h_base + j * s_per:
                                             h_base + (j + 1) * s_per],
                                    in_=pw[:, 0:s_per])
                            if ablate != "nodve" and ablate != "pe_wm":
                                # tmp[b, s, r] = w'[b, s, r] * u[b, r]
                                tmp = tmpp.tile([128, wm_n],
                                                BF if tmp_bf else FP32)
                                nc.vector.tensor_tensor(
                                    out=tmp[:, :].rearrange("p (s r) -> p s r", r=R),
                                    in0=pw[:, :].rearrange("p (s r) -> p s r", r=R),
                                    in1=u_sb[:, :].unsqueeze(1)
                                    .broadcast_to([128, s_per, R]),
                                    op=mybir.AluOpType.mult)
                                # h[b, jc*s_per + s] = sum_r tmp[b, s, r]
                                import contextlib as _ctl
                                _lp = (nc.allow_low_precision(
                                    "h is cast to bf16 before the V GEMM "
                                    "anyway; rounding at the reduce is "
                                    "equivalent") if h_bf
                                    else _ctl.nullcontext())
                                with _lp:
                                    nc.vector.tensor_reduce(
                                        out=h_sb[:, h_base + j * s_per:
                                                 h_base + (j + 1) * s_per],
                                        in_=tmp[:, :].rearrange("p (s r) -> p s r", r=R),
                                        axis=mybir.AxisListType.X,
                                        op=mybir.AluOpType.add)
                            for _ in range(wm_n // 512):
                                emit_slot()
                            if j == 0 or j == nj - 1:
                                emit_slot()
                                emit_slot()
                        assert not pending, "tail did not fit in slot budget"
                        if v_pair and ablate is None:
                            # paired-bt tail: one [128,128] transpose covers
                            # two bts; its output lands ht(bt0) on partitions
                            # 0..63 and ht(bt1) on 64..127, so the V GEMMs run
                            # pairwise-concurrent in disjoint PE row groups.
                            if bt % 2 == 0:
                                continue
                            fns = make_pair_tail(l, bt - 1, bt, hpair,
                                                 xt_next)
                            if pair_defer:
                                pending_pair.extend(fns)
                            else:
                                for fn in fns:
                                    fn()
                            continue
                        if v_batch and ablate is None:
                            # transpose h into the 4-tile group's shared hT
                            # operand; run the V GEMM once per group at
                            # n=512 so LDWEIGHTS amortizes.
                            if bt % 4 == 0:
                                ht4 = ht4p.tile([128 if v_pack else R, 512],
                                                BF, tag="ht4")
                            pt = ptp.tile([R, 128], FP32)
                            nc.tensor.transpose(pt[:, :], h_sb[:, :], ident[:, :])
                            nc.scalar.copy(
                                ht4[0:R, (bt % 4) * 128:(bt % 4 + 1) * 128],
                                pt[:, :])
                            if v_pack:
                                # second copy on partitions 64..127 feeds the
                                # row-group-packed partner matmul
                                nc.scalar.copy(
                                    ht4[R:128, (bt % 4) * 128:(bt % 4 + 1) * 128],
                                    pt[:, :])
                            if bt % 4 == 3:
                                g0 = (bt - 3) * 128
                                if l < 2 and v_pack:
                                    for ocp in range(NOC // 2):
                                        oc0, oc1 = 2 * ocp, 2 * ocp + 1
                                        po0 = pop.tile([128, 512], FP32, tag="po")
                                        po1 = pop.tile([128, 512], FP32, tag="po")
                                        nc.tensor.matmul(
                                            po0[:, :],
                                            vt2_sb[l][0:R, oc0 * 128:(oc0 + 1) * 128],
                                            ht4[0:R, :], start=True, stop=True)
                                        nc.tensor.matmul(
                                            po1[:, :],
                                            vt2_sb[l][R:128, oc1 * 128:(oc1 + 1) * 128],
                                            ht4[R:128, :], start=True, stop=True)
                                        for oc, po in ((oc0, po0), (oc1, po1)):
                                            dst = xt_next[:, oc * BL + g0:
                                                          oc * BL + g0 + 512]
                                            if include_b01:
                                                nc.scalar.activation(
                                                    dst, po[:, :],
                                                    mybir.ActivationFunctionType.Relu,
                                                    bias=b01_sb[l][:, oc:oc + 1],
                                                    scale=1.0)
                                            elif oc % 2 == 1:
                                                # split relus across ACT and DVE
                                                nc.vector.tensor_scalar_max(
                                                    dst, po[:, :], 0.0)
                                            else:
                                                nc.scalar.activation(
                                                    dst, po[:, :],
                                                    mybir.ActivationFunctionType.Relu)
                                elif l < 2:
                                    for oc in range(NOC):
                                        po = pop.tile([128, 512], FP32, tag="po")
                                        nc.tensor.matmul(
                                            po[:, :],
                                            vt_sb[l][:, oc * 128:(oc + 1) * 128],
                                            ht4[:, :], start=True, stop=True)
                                        if include_b01:
                                            nc.scalar.activation(
                                                xt_next[:, oc * BL + g0:
                                                        oc * BL + g0 + 512],
                                                po[:, :],
                                                mybir.ActivationFunctionType.Relu,
                                                bias=b01_sb[l][:, oc:oc + 1],
                                                scale=1.0)
                                        else:
                                            nc.scalar.activation(
                                                xt_next[:, oc * BL + g0:
                                                        oc * BL + g0 + 512],
                                                po[:, :],
                                                mybir.ActivationFunctionType.Relu)
                                elif v_pack:
                                    assert not include_b2
                                    for i4 in range(4):
                                        o_sb = osbp.tile([128, D], FP32)
                                        po0 = pop.tile([128, 512], FP32, tag="po")
                                        po1 = pop.tile([128, 512], FP32, tag="po")
                                        nc.tensor.matmul(
                                            po0[:, :],
                                            ht4[0:R, i4 * 128:(i4 + 1) * 128],
                                            vt2_sb[l][0:R, 0:512],
                                            start=True, stop=True)
                                        nc.tensor.matmul(
                                            po1[:, :],
                                            ht4[R:128, i4 * 128:(i4 + 1) * 128],
                                            vt2_sb[l][R:128, 512:1024],
                                            start=True, stop=True)
                                        nc.scalar.copy(o_sb[:, 0:512], po0[:, :])
                                        nc.vector.tensor_copy(
                                            out=o_sb[:, 512:1024], in_=po1[:, :])
                                        row0 = (bt - 3 + i4) * 128
                                        nc.sync.dma_start(
                                            out=out_d[row0:row0 + 128, :],
                                            in_=o_sb[:, :])
                                else:
                                    for i4 in range(4):
                                        o_sb = osbp.tile([128, D], FP32)
                                        for half in range(2):
                                            osl = slice(half * 512,
                                                        (half + 1) * 512)
                                            po = pop.tile([128, 512], FP32,
                                                          tag="po")
                                            nc.tensor.matmul(
                                                po[:, :],
                                                ht4[:, i4 * 128:(i4 + 1) * 128],
                                                vt_sb[l][:, osl],
                                                start=True, stop=not include_b2)
                                            if include_b2:
                                                nc.tensor.matmul(
                                                    po[:, :], ones_bf[:, :],
                                                    b2_sb[:, osl],
                                                    start=False, stop=True)
                                            nc.scalar.copy(o_sb[:, osl],
                                                           po[:, :])
                                        row0 = (bt - 3 + i4) * 128
                                        nc.sync.dma_start(
                                            out=out_d[row0:row0 + 128, :],
                                            in_=o_sb[:, :])
                            continue
                        if ablate == "pe_wm":
                            if l == 2 and bt == NBT - 1:
                                # keep the output write so the graph has one
                                nc.sync.dma_start(
                                    out=out_d[bsl, :],
                                    in_=xt_b[:, :].bitcast(FP32)[:, 0:D])
                            continue
                        pending = make_tail(l, bt, h_sb, xt_next, bsl)
                        if not pipeline:
                            for fn in pending:
                                fn()
                            pending = []
                    if xt_next is not None:
                        xt_l = xt_next

                while pending_pair:
                    pending_pair.pop(0)()
                for fn in pending:
                    fn()
                pending = []
    nc.compile()
    if dedup_ldw:
        n = dedup_ldweights(nc)
        print(f"dedup_ldweights: removed {n}")
    if batch_updates:
        n = batch_mm_updates(nc, every=batch_updates)
        print(f"batch_mm_updates: batched {n}")
    return nc


# ---------------------------------------------------------------------------
# host side
# ---------------------------------------------------------------------------

def _prep_static(Wm, bm, U, V, b):
    """Host-side layout prep of one layer's replicated params."""
    Wm = np.asarray(Wm, dtype=np.float32)
    # rows j' = s*64 + r  <->  original j = r*64 + s ; then transpose -> [k, j']
    wmt = np.ascontiguousarray(
        Wm.reshape(R, R, D).transpose(1, 0, 2).reshape(R * R, D).T).astype(BF16)
    utm = np.ascontiguousarray(np.asarray(U, dtype=np.float32).T).astype(BF16)
    vtm = np.ascontiguousarray(np.asarray(V, dtype=np.float32).T).astype(BF16)
    return wmt, utm, vtm


_CACHE = {}


def _get_compiled(flags):
    if flags not in _CACHE:
        _CACHE[flags] = build_apg(*flags)
    return _CACHE[flags]


def _make_in_maps(x, layers, flags, reps):
    include_bm, include_b01, include_b2 = flags
    x = np.asarray(x, dtype=np.float32)
    shared = {}
    for l, (Wm, bm, U, V, b) in enumerate(layers):
        wmt, utm, vtm = _prep_static(Wm, bm, U, V, b)
        shared[f"wmt{l}"] = wmt
        shared[f"ut{l}"] = utm
        shared[f"vt{l}"] = vtm
        if include_bm:
            shared[f"bmr{l}"] = np.asarray(bm, np.float32).reshape(R, R).T \
                .reshape(1, R * R).astype(BF16)
        if include_b01 and l < 2:
            shared[f"b{l}c"] = np.ascontiguousarray(
                np.asarray(b, np.float32).reshape(NOC, 128).T)
        if include_b2 and l == 2:
            shared["b2r"] = np.asarray(b, np.float32).reshape(1, D).astype(BF16)
    shared["reps"] = np.array([[reps]], dtype=np.uint32)
    in_maps = []
    for i in range(NCORES):
        m = dict(shared)
        xs = x[i * BL:(i + 1) * BL, :]
        m["xt"] = np.ascontiguousarray(xs.T).astype(BF16)
        in_maps.append(m)
    return in_maps


_RUNNER_CACHE = {}


def _get_runner(flags):
    """Jit-once PJRT runner for the compiled module (same execution path as
    bass_utils.run_bass_kernel_spmd's axon redirect through bass2jax, but
    cached so repeat kernel() calls skip re-trace/re-compile)."""
    if flags in _RUNNER_CACHE:
        return _RUNNER_CACHE[flags]
    import jax
    from jax.sharding import Mesh, PartitionSpec, NamedSharding
    from jax.experimental.shard_map import shard_map
    from concourse import bass2jax

    nc = _get_compiled(flags)
    bass2jax.install_neuronx_cc_hook()
    partition_name = nc.partition_id_tensor.name if nc.partition_id_tensor else None
    in_names, out_names, out_avals, zero_outs = [], [], [], []
    for alloc in nc.m.functions[0].allocations:
        if not isinstance(alloc, mybir.MemoryLocationSet):
            continue
        name = alloc.memorylocations[0].name
        if alloc.kind == "ExternalInput":
            if name != partition_name:
                in_names.append(name)
        elif alloc.kind == "ExternalOutput":
            out_names.append(name)
            shape = tuple(alloc.tensor_shape)
            dtype = mybir.dt.np(alloc.dtype)
            out_avals.append(jax.core.ShapedArray(shape, dtype))
            zero_outs.append(np.zeros(shape, dtype))
    n_params = len(in_names)
    all_in_names = list(in_names) + list(out_names)
    if partition_name is not None:
        all_in_names.append(partition_name)

    def _body(*args):
        operands = list(args)
        if partition_name is not None:
            operands = operands + [bass2jax.partition_id_tensor()]
        outs = bass2jax._bass_exec_p.bind(
            *operands, out_avals=tuple(out_avals), in_names=tuple(all_in_names),
            out_names=tuple(out_names), lowering_input_output_aliases=(),
            sim_require_finite=True, sim_require_nnan=True, nc=nc)
        return tuple(outs)

    devices = jax.devices()[:NCORES]
    mesh = Mesh(np.asarray(devices), ("core",))
    in_specs = (PartitionSpec("core"),) * (n_params + len(out_names))
    out_specs = (PartitionSpec("core"),) * len(out_names)
    fn = jax.jit(shard_map(_body, mesh=mesh, in_specs=in_specs,
                           out_specs=out_specs, check_rep=False))
    sh = NamedSharding(mesh, PartitionSpec("core"))

    # Outputs are constant zero-filled donor buffers — upload once.
    zero_dev = [jax.device_put(np.concatenate([z] * NCORES, axis=0), sh)
                for z in zero_outs]
    dev_cache = {}

    def _put_cached(name, arr):
        import hashlib
        key = (name, arr.shape, arr.dtype.str,
               hashlib.blake2b(np.ascontiguousarray(arr).tobytes(),
                               digest_size=16).hexdigest())
        if key not in dev_cache:
            if len(dev_cache) > 64:
                dev_cache.clear()
            dev_cache[key] = jax.device_put(arr, sh)
        return dev_cache[key]

    def run(in_maps):
        dev = [_put_cached(name,
                           np.concatenate([np.asarray(m[name]) for m in in_maps],
                                          axis=0))
               for name in in_names]
        outs = fn(*(dev + zero_dev))
        jax.block_until_ready(outs)
        return {name: np.asarray(outs[i]) for i, name in enumerate(out_names)}

    _RUNNER_CACHE[flags] = run
    return run


def kernel(x, Wm0, bm0, U0, V0, b0, Wm1, bm1, U1, V1, b1,
           Wm2, bm2, U2, V2, b2):
    layers = [(Wm0, bm0, U0, V0, b0), (Wm1, bm1, U1, V1, b1),
              (Wm2, bm2, U2, V2, b2)]
    flags = (
        any(np.any(np.asarray(t[1], np.float32)) for t in layers),
        any(np.any(np.asarray(t[4], np.float32)) for t in layers[:2]),
        bool(np.any(np.asarray(layers[2][4], np.float32))),
    )
    run = _get_runner(flags)
    in_maps = _make_in_maps(x, layers, flags, reps=1)
    res = None
    for attempt in range(3):
        try:
            res = run(in_maps)
            # transient device flakes can also corrupt silently (NaNs in
            # the output); the math here is all-finite by construction
            if np.isfinite(res["out"]).all():
                break
        except Exception:
            if attempt == 2:
                raise
        # retry on fresh runner + device buffers
        _RUNNER_CACHE.pop(flags, None)
        run = _get_runner(flags)
    # res["out"] is the concatenation of the 8 per-core [BL, D] shards
    return np.ascontiguousarray(res["out"]).astype(np.float32)

